# revision 1
# baseline (speedup 1.0000x reference)
"""TV-Chambolle denoise (weight=0.1, eps=2e-4, n_iter_max=200) on 8 Trainium2
NeuronCores via Bass/Tile.

Sharding: embarrassingly parallel over channels — core c solves channel c%3
(cores 3-7 run duplicates; host reads cores 0-2).

Layout per channel: 512x512 fp32 image in "strip" layout [128, 4*512]:
partition p holds rows 4p..4p+3 contiguously (C-order reshape(128, 2048)).
H-direction stencil shifts are free-dim offsets for 3/4 of rows; the 127
strip-boundary rows use SBUF->SBUF DMA halo copies with partition remap.

Early stopping: the reference freezes its state once |E_prev-E| < eps*E_init.
On device this is done with per-partition scalar tau_eff = tau*(1-done) where
done incorporates the CURRENT iteration's convergence flag: p then freezes at
the conv iteration i*, so t = img + div(p_{i*}) equals the reference's output
with no plane-level selects. The kernel runs K=25 iterations per launch and
outputs (t, p0, p1, scalars); the host relaunches (up to 200 total iterations)
only if some channel has not converged. The reference input converges at
iteration 21, so one launch suffices.
"""
import sys
if '/opt/trn_rl_repo' not in sys.path:
    sys.path.insert(0, '/opt/trn_rl_repo')

import numpy as np

F32_EPS = 2e-4
WEIGHT = 0.1
TAU = 0.25
P, J, W = 128, 4, 512
FREE = J * W
K_CHUNK = 25
N_ITER_MAX = 200
N_CORES = 8

_NC = None
LAST_RESULTS = []


def _build():
    import concourse.bacc as bacc
    import concourse.tile as tile
    import concourse.mybir as mybir
    from concourse import bass_isa
    from contextlib import ExitStack

    F32 = mybir.dt.float32
    ALU = mybir.AluOpType
    ACTF = mybir.ActivationFunctionType
    K = K_CHUNK

    nc = bacc.Bacc('TRN2', target_bir_lowering=False, debug=False)

    img_d = nc.declare_dram_parameter("img", [P, FREE], F32, isOutput=False)
    p0_d = nc.declare_dram_parameter("p0_in", [P, FREE], F32, isOutput=False)
    p1_d = nc.declare_dram_parameter("p1_in", [P, FREE], F32, isOutput=False)
    scal_d = nc.declare_dram_parameter("scal_in", [P, 4], F32, isOutput=False)
    sd_d = nc.declare_dram_parameter("Sd", [P, P], F32, isOutput=False)
    su_d = nc.declare_dram_parameter("Su", [P, P], F32, isOutput=False)
    out_d = nc.declare_dram_parameter("out_t", [P, FREE], F32, isOutput=True)
    p0o_d = nc.declare_dram_parameter("p0_out", [P, FREE], F32, isOutput=True)
    p1o_d = nc.declare_dram_parameter("p1_out", [P, FREE], F32, isOutput=True)
    scalo_d = nc.declare_dram_parameter("scal_out", [P, 4], F32, isOutput=True)

    with tile.TileContext(nc) as tc, ExitStack() as ctx:
        pool = ctx.enter_context(tc.tile_pool(name="st", bufs=1))
        pspool = ctx.enter_context(tc.tile_pool(name="ps", bufs=1, space="PSUM"))

        def T(name, shape=(P, FREE)):
            return pool.tile(list(shape), F32, name=name, tag=name)

        img = T("img_t"); p0 = T("p0"); p1 = T("p1")
        dneg = T("dneg"); Bp = T("Bp"); t = T("t")
        g0 = T("g0"); g1 = T("g1")
        sq0 = T("sq0"); n2 = T("n2")
        denom = T("den"); r = T("r"); rs = T("rs")
        u0 = T("u0"); u1 = T("u1")
        scr = T("scr")
        Sd = T("Sd_t", (P, P)); Su = T("Su_t", (P, P))
        ones_col = T("ones_col", (P, 1)); ones_row = T("ones_row", (1, P))
        esc = T("esc", (1, 1))
        halo_p = pspool.tile([P, W], F32, name="halo_p", tag="halo_p")
        halo_t = pspool.tile([P, W], F32, name="halo_t", tag="halo_t")
        e1_ps = pspool.tile([1, 1], F32, name="e1_ps", tag="e1_ps")
        eb_ps = pspool.tile([P, 1], F32, name="eb_ps", tag="eb_ps")
        scal = T("scal", (P, 4))
        Ed = T("Ed", (P, 1)); En = T("En", (P, 1)); c_ = T("c", (P, 1))
        Es = T("Es", (P, 1)); dE = T("dE", (P, 1)); th = T("th", (P, 1))
        conv = T("conv", (P, 1)); nfirst = T("nf", (P, 1))
        notdone = T("nd", (P, 1)); s_u = T("s_u", (P, 1)); s_ow = T("s_ow", (P, 1))
        tmp1 = T("tmp1", (P, 1)); tmp2 = T("tmp2", (P, 1))

        E_prev = scal[:, 0:1]; E_init = scal[:, 1:2]
        done = scal[:, 2:3]; first = scal[:, 3:4]

        nc.sync.dma_start(img[:], img_d.ap())
        nc.sync.dma_start(p0[:], p0_d.ap())
        nc.sync.dma_start(p1[:], p1_d.ap())
        nc.sync.dma_start(scal[:], scal_d.ap())
        nc.sync.dma_start(Sd[:], sd_d.ap())
        nc.sync.dma_start(Su[:], su_d.ap())

        nc.vector.memset(g0[:], 0.0)
        nc.vector.memset(g1[:], 0.0)
        nc.vector.memset(ones_col[:], 1.0)
        nc.vector.memset(ones_row[:], 1.0)
        nc.vector.tensor_scalar(nfirst[:], first[:], -1.0, 1.0, ALU.mult, ALU.add)
        # halo_p[m,:] = p0[m-1, last row block] via shift matmul (row 0 = 0)
        nc.tensor.matmul(halo_p[:], Sd[:], p0[:, 3 * W:4 * W], start=True, stop=True)

        def v3(ap):
            return ap.rearrange("p (j w) -> p j w", w=W)

        for j in range(K):
            # B' = p1 - shiftW(p1)  (GPSIMD, overlaps the previous iteration's tail)
            Bp3 = v3(Bp[:]); p13 = v3(p1[:])
            nc.gpsimd.tensor_copy(Bp3[:, :, 0:1], p13[:, :, 0:1])
            nc.gpsimd.tensor_tensor(Bp3[:, :, 1:W], p13[:, :, 1:W], p13[:, :, 0:W - 1], ALU.subtract)

            # A = p0 - shiftH(p0) into dneg (DVE); halo term from PSUM (PE matmul)
            nc.vector.tensor_copy(dneg[:], p0[:])
            d3 = v3(dneg[:]); p03 = v3(p0[:])
            nc.vector.tensor_tensor(d3[:, 1:4, :], d3[:, 1:4, :], p03[:, 0:3, :], ALU.subtract)
            nc.vector.tensor_tensor(d3[:, 0, :], d3[:, 0, :], halo_p[:, :], ALU.subtract)
            nc.vector.tensor_add(dneg[:], dneg[:], Bp[:])

            # t = img - dneg  (dneg == -div(p))
            nc.vector.tensor_sub(t[:], img[:], dneg[:])
            # halo_t[m,:] = t[m+1, first row block] via shift matmul (row 127 = 0)
            nc.tensor.matmul(halo_t[:], Su[:], t[:, 0:W], start=True, stop=True)

            # Ed = sum(dneg^2) per partition (ACT)
            nc.scalar.activation(scr[:], dneg[:], ACTF.Square, accum_out=Ed[:])

            # gradients: g0 on DVE (halo from PSUM), g1 on GPSIMD
            t3 = v3(t[:]); g03 = v3(g0[:]); g13 = v3(g1[:])
            nc.vector.tensor_tensor(g03[:, 0:3, :], t3[:, 1:4, :], t3[:, 0:3, :], ALU.subtract)
            nc.vector.tensor_tensor(g03[0:127, 3, :], halo_t[0:127, :], t3[0:127, 3, :], ALU.subtract)
            nc.gpsimd.tensor_tensor(g13[:, :, 0:W - 1], t3[:, :, 1:W], t3[:, :, 0:W - 1], ALU.subtract)

            # n2 = g0^2 + g1^2 (squares on ACT, add on DVE); norm = sqrt(n2) + En
            nc.scalar.activation(sq0[:], g0[:], ACTF.Square)
            nc.scalar.activation(n2[:], g1[:], ACTF.Square)
            nc.vector.tensor_add(n2[:], n2[:], sq0[:])
            nc.scalar.activation(n2[:], n2[:], ACTF.Sqrt, accum_out=En[:])
            norm = n2

            # denom with CONSTANT scale; freeze applied to r afterwards.
            nc.scalar.activation(denom[:], norm[:], ACTF.Identity, bias=1.0,
                                 scale=float(TAU / WEIGHT))
            # recips FIRST in DVE program order (DVE is in-order; these must not
            # queue behind the convergence-scalar chain)
            nc.vector.reciprocal_approx_accurate(r[:], denom[:], rs[:])

            # E chain; E kept raw (x size) — scale-invariant test. Cross-partition
            # reduce + broadcast on the idle PE (GpSimd sem-wake is ~7us).
            nc.vector.scalar_tensor_tensor(c_[:], En[:], WEIGHT, Ed[:], ALU.mult, ALU.add)
            nc.tensor.matmul(e1_ps[:], c_[:], ones_col[:], start=True, stop=True)
            nc.vector.tensor_copy(esc[:], e1_ps[:])
            nc.tensor.matmul(eb_ps[:], ones_row[:], esc[:], start=True, stop=True)
            nc.vector.tensor_copy(Es[:], eb_ps[:])
            if j == 0:
                nc.vector.tensor_mul(tmp1[:], Es[:], first[:])
                nc.vector.tensor_mul(tmp2[:], E_init, nfirst[:])
                nc.vector.tensor_add(E_init, tmp1[:], tmp2[:])
            nc.vector.tensor_sub(dE[:], E_prev, Es[:])
            # |dE| < th  <=>  dE^2 < th^2  (th >= 0) — avoids an ACT round-trip
            nc.vector.tensor_mul(dE[:], dE[:], dE[:])
            nc.vector.tensor_scalar(th[:], E_init, float(F32_EPS), None, ALU.mult)
            nc.vector.tensor_mul(th[:], th[:], th[:])
            nc.vector.tensor_tensor(conv[:], dE[:], th[:], ALU.is_lt)
            nc.vector.tensor_tensor(done, done, conv[:], ALU.max)
            nc.vector.tensor_copy(E_prev, Es[:])
            nc.vector.tensor_scalar(notdone[:], done, -1.0, 1.0, ALU.mult, ALU.add)
            nc.vector.tensor_scalar(s_u[:], notdone[:], float(-TAU), None, ALU.mult)

            # r_eff = r*notdone + done (exactly 1.0 when done; exact freeze)
            nc.vector.tensor_scalar(r[:], r[:], notdone[:], done, ALU.mult, ALU.add)

            # p update; p1 first so next iteration's GPSIMD W-shift starts early
            nc.vector.scalar_tensor_tensor(u1[:], g1[:], s_u[:], p1[:], ALU.mult, ALU.add)
            nc.vector.tensor_mul(p1[:], u1[:], r[:])
            nc.vector.scalar_tensor_tensor(u0[:], g0[:], s_u[:], p0[:], ALU.mult, ALU.add)
            nc.vector.tensor_mul(p0[:], u0[:], r[:])

            if j + 1 < K:
                nc.tensor.matmul(halo_p[:], Sd[:], p0[:, 3 * W:4 * W], start=True, stop=True)

        nc.sync.dma_start(out_d.ap(), t[:])
        nc.sync.dma_start(p0o_d.ap(), p0[:])
        nc.sync.dma_start(p1o_d.ap(), p1[:])
        nc.sync.dma_start(scalo_d.ap(), scal[:])

    nc.compile()
    return nc


def _get_nc():
    global _NC
    if _NC is None:
        _NC = _build()
    return _NC


def kernel(img: np.ndarray) -> np.ndarray:
    from concourse.bass_utils import run_bass_kernel_spmd

    assert img.shape == (3, 512, 512) and img.dtype == np.float32
    nc = _get_nc()
    del LAST_RESULTS[:]

    core_ids = list(range(N_CORES))
    p0s = [np.zeros((P, FREE), np.float32) for _ in core_ids]
    p1s = [np.zeros((P, FREE), np.float32) for _ in core_ids]
    scals = []
    for c in core_ids:
        s = np.zeros((P, 4), np.float32)
        s[:, 3] = 1.0  # first chunk
        scals.append(s)
    imgs = [np.ascontiguousarray(img[c % 3].reshape(P, FREE)) for c in core_ids]
    Sd = np.eye(P, k=1, dtype=np.float32)   # halo_p[m] = p0[m-1]
    Su = np.eye(P, k=-1, dtype=np.float32)  # halo_t[m] = t[m+1]

    iters = 0
    outs = None
    while iters < N_ITER_MAX:
        in_maps = [
            {"img": imgs[c], "p0_in": p0s[c], "p1_in": p1s[c], "scal_in": scals[c],
             "Sd": Sd, "Su": Su}
            for c in core_ids
        ]
        res = run_bass_kernel_spmd(nc, in_maps, core_ids)
        LAST_RESULTS.append(res)
        iters += K_CHUNK
        outs = res.results
        if all(outs[c]["scal_out"][0, 2] > 0.5 for c in range(3)):
            break
        for c in core_ids:
            p0s[c] = outs[c]["p0_out"]
            p1s[c] = outs[c]["p1_out"]
            s = outs[c]["scal_out"].copy()
            s[:, 3] = 0.0  # no longer the first chunk
            scals[c] = s

    result = np.empty((3, 512, 512), np.float32)
    for c in range(3):
        result[c] = outs[c]["out_t"].reshape(512, 512)
    return result



# revision 2
# speedup vs baseline: 2.5375x; 2.5375x over previous
"""TV-Chambolle denoise (weight=0.1, eps=2e-4, n_iter_max=200) on 8 Trainium2
NeuronCores via Bass/Tile — v2.

Strategy vs v1 (1.1ms):
- Unconditional iterations: the reference's early-stop freeze is emulated on
  the HOST. The device runs K=26 plain Chambolle iterations, accumulates the
  per-iteration energy partial sums Ed_j = sum(d^2), En_j = sum(norm) via ACT
  accum_out, and streams the iterate t_j (j >= J_LO) to DRAM. The host finds
  the freeze iteration i* = first j>=1 with |E_{j-1}-E_j| < eps*E_0 and picks
  t_{i*}. (out_final = img + div(p_{i*}) = t computed during step i*.)
  This removes the long serialized per-iteration convergence chain.
- fp16 tiles: 2x DVE throughput on tensor_tensor (2x_1P mode).
- PE computes the strip-boundary (partition-crossing) stencil blocks directly
  into PSUM via paired accumulating matmuls (I@x - Shift@y), ACT copies them
  out — no DVE halo ops.
- 6 useful cores: channel c is W-split across cores 2c (cols 0..287 of 512,
  owns 0..255) and 2c+1 (cols 224..511, owns 256..511). The 32 ghost columns
  make each half's owned region exact for >= 32 iterations with ZERO
  inter-core communication (1 col/iteration dependency horizon). Cores 6,7
  run duplicate work (ignored).

Layout per core: [128, 4*288] fp16 strip layout — partition p holds image
rows 4p..4p+3 of its 288-col slice.
"""
import sys
if '/opt/trn_rl_repo' not in sys.path:
    sys.path.insert(0, '/opt/trn_rl_repo')

import numpy as np

EPS = 2e-4
WEIGHT = 0.1
TAU = 0.25
C_TW = TAU / WEIGHT

P = 128
J = 4
WT = 288          # per-core tile width (cols): 256 owned + 32 ghost
OWN = 256
GHOST = 32
FREE = J * WT
K = 26            # unconditional iterations per launch
J_LO = 16         # stream t_j for j in [J_LO, K)
NSNAP = K - J_LO
N_CORES = 8
H = 512

_NC = None
LAST_RESULTS = []


def _build():
    import concourse.bacc as bacc
    import concourse.tile as tile
    import concourse.mybir as mybir
    from contextlib import ExitStack

    F16 = mybir.dt.float16
    F32 = mybir.dt.float32
    ALU = mybir.AluOpType
    ACTF = mybir.ActivationFunctionType

    nc = bacc.Bacc('TRN2', target_bir_lowering=False, debug=False)

    img_d = nc.declare_dram_parameter("img", [P, FREE], F16, isOutput=False)
    ia_d = nc.declare_dram_parameter("Ia", [P, P], F16, isOutput=False)
    sdm_d = nc.declare_dram_parameter("Sdm", [P, P], F16, isOutput=False)
    sup_d = nc.declare_dram_parameter("Sup", [P, P], F16, isOutput=False)
    inz_d = nc.declare_dram_parameter("Inz", [P, P], F16, isOutput=False)
    ts_d = nc.declare_dram_parameter("ts", [P, NSNAP * FREE], F16, isOutput=True)
    eden_d = nc.declare_dram_parameter("eden", [P, 2 * K], F32, isOutput=True)

    with tile.TileContext(nc) as tc, ExitStack() as ctx:
        pool = ctx.enter_context(tc.tile_pool(name="st", bufs=1))
        pspool = ctx.enter_context(tc.tile_pool(name="ps", bufs=1, space="PSUM"))

        def T(name, shape=(P, FREE), dt=F16):
            return pool.tile(list(shape), dt, name=name, tag=name)

        img = T("img_t"); p0 = T("p0"); p1 = T("p1")
        Bp = T("Bp"); dneg = T("dneg"); tscr = T("tscr")
        g0 = T("g0"); g1 = T("g1")
        sq0 = T("sq0"); n2 = T("n2"); scr = T("scr")
        r = T("r"); u0 = T("u0"); u1 = T("u1")
        s32 = T("s32", dt=F32); d32 = T("d32", dt=F32); rf = T("rf", dt=F32)
        Ia = T("Ia_t", (P, P)); Sdm = T("Sdm_t", (P, P))
        Sup = T("Sup_t", (P, P)); Inz = T("Inz_t", (P, P))
        eden = T("eden", (P, 2 * K), F32)
        snaps = [T(f"snap{i}") for i in range(NSNAP)]
        psum0 = pspool.tile([P, WT], F32, name="psum0", tag="psum0")
        psum3 = pspool.tile([P, WT], F32, name="psum3", tag="psum3")

        nc.sync.dma_start(img[:], img_d.ap())
        nc.sync.dma_start(Ia[:], ia_d.ap())
        nc.sync.dma_start(Sdm[:], sdm_d.ap())
        nc.sync.dma_start(Sup[:], sup_d.ap())
        nc.sync.dma_start(Inz[:], inz_d.ap())

        nc.vector.memset(p0[:], 0.0)
        nc.vector.memset(p1[:], 0.0)
        nc.vector.memset(g1[:], 0.0)   # col WT-1 must stay 0 (never written in loop)

        def v3(ap):
            return ap.rearrange("p (j w) -> p j w", w=WT)

        for j in range(K):
            t = snaps[j - J_LO] if j >= J_LO else tscr
            p03 = v3(p0[:]); p13 = v3(p1[:]); d3 = v3(dneg[:])
            t3 = v3(t[:]); g03 = v3(g0[:]); g13 = v3(g1[:]); Bp3 = v3(Bp[:])

            # strip-boundary block of (p0 - shiftH p0): psum0 = I@p0_s0 - Sd@p0_s3
            nc.tensor.matmul(psum0[:], Ia[:], p0[:, 0:WT], start=True, stop=False)
            nc.tensor.matmul(psum0[:], Sdm[:], p0[:, 3 * WT:], start=False, stop=True)

            # Bp = p1 - shiftW(p1)  (GPSIMD)
            nc.gpsimd.tensor_copy(Bp3[:, :, 0:1], p13[:, :, 0:1])
            nc.gpsimd.tensor_tensor(Bp3[:, :, 1:WT], p13[:, :, 1:WT],
                                    p13[:, :, 0:WT - 1], ALU.subtract)

            # interior of (p0 - shiftH p0)
            nc.vector.tensor_tensor(d3[:, 1:4, :], p03[:, 1:4, :],
                                    p03[:, 0:3, :], ALU.subtract)
            # boundary block from PSUM
            nc.scalar.activation(d3[:, 0, :], psum0[:], ACTF.Copy)
            # dneg = (p0 - shiftH p0) + (p1 - shiftW p1)
            nc.vector.tensor_add(dneg[:], dneg[:], Bp[:])

            # Ed_j = sum(dneg^2)
            nc.scalar.activation(scr[:], dneg[:], ACTF.Square,
                                 accum_out=eden[:, 2 * j:2 * j + 1])

            # t = img - dneg
            nc.vector.tensor_sub(t[:], img[:], dneg[:])

            # strip-boundary block of g0: psum3 = Su@t_s0 - Iz@t_s3
            nc.tensor.matmul(psum3[:], Sup[:], t[:, 0:WT], start=True, stop=False)
            nc.tensor.matmul(psum3[:], Inz[:], t[:, 3 * WT:], start=False, stop=True)

            # g0 interior; boundary from PSUM
            nc.vector.tensor_tensor(g03[:, 0:3, :], t3[:, 1:4, :],
                                    t3[:, 0:3, :], ALU.subtract)
            nc.scalar.activation(g03[:, 3, :], psum3[:], ACTF.Copy)

            # g1 = shiftW^-1(t) - t  (GPSIMD; col WT-1 stays 0)
            nc.gpsimd.tensor_tensor(g13[:, :, 0:WT - 1], t3[:, :, 1:WT],
                                    t3[:, :, 0:WT - 1], ALU.subtract)

            # n2 = g0^2 + g1^2
            nc.scalar.activation(sq0[:], g0[:], ACTF.Square)
            nc.scalar.activation(n2[:], g1[:], ACTF.Square)
            nc.vector.tensor_add(n2[:], n2[:], sq0[:])

            # norm = sqrt(n2); En_j = sum(norm)
            nc.scalar.activation(s32[:], n2[:], ACTF.Sqrt,
                                 accum_out=eden[:, 2 * j + 1:2 * j + 2])

            # r = 1 / (1 + (tau/weight)*norm)
            nc.vector.tensor_scalar(d32[:], s32[:], float(C_TW), 1.0,
                                    ALU.mult, ALU.add)
            nc.vector.reciprocal_approx_fast(rf[:], d32[:])
            nc.vector.tensor_copy(r[:], rf[:])

            # p updates; p1 first so next iteration's GPSIMD W-shift starts early
            nc.vector.scalar_tensor_tensor(u1[:], g1[:], float(-TAU), p1[:],
                                           ALU.mult, ALU.add)
            nc.vector.tensor_mul(p1[:], u1[:], r[:])
            nc.vector.scalar_tensor_tensor(u0[:], g0[:], float(-TAU), p0[:],
                                           ALU.mult, ALU.add)
            nc.vector.tensor_mul(p0[:], u0[:], r[:])

            if j >= J_LO:
                nc.sync.dma_start(ts_d.ap()[:, (j - J_LO) * FREE:(j - J_LO + 1) * FREE],
                                  t[:])

        nc.sync.dma_start(eden_d.ap(), eden[:])

    nc.compile()
    return nc


def _get_nc():
    global _NC
    if _NC is None:
        _NC = _build()
    return _NC


def _host_reference_fallback(img):
    """Exact CPU port of the reference (incl. freeze); only used if the
    device E-sequence fails to locate i* inside [J_LO, K)."""
    out = np.empty_like(img)
    for c in range(img.shape[0]):
        image = img[c].astype(np.float64)
        Hh, Ww = image.shape
        tau = 0.25
        p = np.zeros((2, Hh, Ww))
        o = image.copy()
        E_init = None
        E_prev = None
        for i in range(200):
            d = -p.sum(0)
            d[1:, :] += p[0, :-1, :]
            d[:, 1:] += p[1, :, :-1]
            o = image + d
            gg0 = np.zeros_like(o); gg0[:-1] = o[1:] - o[:-1]
            gg1 = np.zeros_like(o); gg1[:, :-1] = o[:, 1:] - o[:, :-1]
            nrm = np.sqrt(gg0 * gg0 + gg1 * gg1)
            E = ((d * d).sum() + WEIGHT * nrm.sum()) / (Hh * Ww)
            if i == 0:
                E_init = E
            elif abs(E_prev - E) < EPS * E_init:
                break
            E_prev = E
            p = (p - tau * np.stack([gg0, gg1])) / (1.0 + C_TW * nrm[None])
        out[c] = o.astype(np.float32)
    return out


def kernel(img: np.ndarray) -> np.ndarray:
    from concourse.bass_utils import run_bass_kernel_spmd

    assert img.shape == (3, 512, 512) and img.dtype == np.float32
    nc = _get_nc()
    del LAST_RESULTS[:]

    Ia = np.eye(P, dtype=np.float16)
    Sdm = (-np.eye(P, k=1)).astype(np.float16)   # psum0[m] -= p0_s3[m-1]
    Sup = np.eye(P, k=-1, dtype=np.float16)      # psum3[m] += t_s0[m+1]
    Inz = (-np.eye(P)).astype(np.float16)
    Inz[P - 1, P - 1] = 0.0                      # g0 row 511 = 0

    # core -> (channel, col range of its 288-wide slice)
    col_lo = [0, H - WT]     # half 0: cols 0..287; half 1: cols 224..511
    core_map = [(c // 2, c % 2) for c in range(6)] + [(0, 0), (1, 0)]

    in_maps = []
    for c in range(N_CORES):
        ch, half = core_map[c]
        lo = col_lo[half]
        sl = np.ascontiguousarray(img[ch][:, lo:lo + WT]).astype(np.float16)
        in_maps.append({
            "img": sl.reshape(P, FREE),
            "Ia": Ia, "Sdm": Sdm, "Sup": Sup, "Inz": Inz,
        })

    res = run_bass_kernel_spmd(nc, in_maps, list(range(N_CORES)))
    LAST_RESULTS.append(res)
    outs = res.results

    result = np.empty((3, 512, 512), np.float32)
    ok = True
    for ch in range(3):
        # E_j from the pair's summed partials (scale-invariant stopping rule)
        ed = (outs[2 * ch]["eden"].astype(np.float64).sum(0)
              + outs[2 * ch + 1]["eden"].astype(np.float64).sum(0))
        E = ed[0::2] + WEIGHT * ed[1::2]
        istar = None
        for jj in range(1, K):
            if abs(E[jj - 1] - E[jj]) < EPS * E[0]:
                istar = jj
                break
        if istar is None or istar < J_LO:
            ok = False
            break
        for half in (0, 1):
            t = outs[2 * ch + half]["ts"][:, (istar - J_LO) * FREE:
                                          (istar - J_LO + 1) * FREE]
            t = t.reshape(H, WT).astype(np.float32)
            if half == 0:
                result[ch][:, 0:OWN] = t[:, 0:OWN]
            else:
                result[ch][:, OWN:H] = t[:, WT - OWN:WT]
    if not ok:
        return _host_reference_fallback(img)
    return result


# revision 6
# speedup vs baseline: 2.8497x; 1.1230x over previous
"""TV-Chambolle denoise (weight=0.1, eps=2e-4, n_iter_max=200) on 8 Trainium2
NeuronCores via Bass/Tile — v2.

Strategy vs v1 (1.1ms):
- Unconditional iterations: the reference's early-stop freeze is emulated on
  the HOST. The device runs K=26 plain Chambolle iterations, accumulates the
  per-iteration energy partial sums Ed_j = sum(d^2), En_j = sum(norm) via ACT
  accum_out, and streams the iterate t_j (j >= J_LO) to DRAM. The host finds
  the freeze iteration i* = first j>=1 with |E_{j-1}-E_j| < eps*E_0 and picks
  t_{i*}. (out_final = img + div(p_{i*}) = t computed during step i*.)
  This removes the long serialized per-iteration convergence chain.
- fp16 tiles: 2x DVE throughput on tensor_tensor (2x_1P mode).
- PE computes the strip-boundary (partition-crossing) stencil blocks directly
  into PSUM via paired accumulating matmuls (I@x - Shift@y), ACT copies them
  out — no DVE halo ops.
- 6 useful cores: channel c is W-split across cores 2c (cols 0..287 of 512,
  owns 0..255) and 2c+1 (cols 224..511, owns 256..511). The 32 ghost columns
  make each half's owned region exact for >= 32 iterations with ZERO
  inter-core communication (1 col/iteration dependency horizon). Cores 6,7
  run duplicate work (ignored).

Layout per core: [128, 4*288] fp16 strip layout — partition p holds image
rows 4p..4p+3 of its 288-col slice.
"""
import sys
if '/opt/trn_rl_repo' not in sys.path:
    sys.path.insert(0, '/opt/trn_rl_repo')

import numpy as np

EPS = 2e-4
WEIGHT = 0.1
TAU = 0.25
C_TW = TAU / WEIGHT

P = 128
J = 4
WT = 288          # per-core tile width (cols): 256 owned + 32 ghost
OWN = 256
GHOST = 32
FREE = J * WT
K = 24            # unconditional iterations per launch
J_LO = 14         # stream t_j for j in [J_LO, K)
NSNAP = K - J_LO
N_CORES = 8
H = 512

_NC = None
LAST_RESULTS = []
DIAG = {}


def _build():
    import concourse.bacc as bacc
    import concourse.tile as tile
    import concourse.mybir as mybir
    from contextlib import ExitStack

    F16 = mybir.dt.float16
    F32 = mybir.dt.float32
    ALU = mybir.AluOpType
    ACTF = mybir.ActivationFunctionType

    nc = bacc.Bacc('TRN2', target_bir_lowering=False, debug=False)

    img_d = nc.declare_dram_parameter("img", [P, FREE], F16, isOutput=False)
    ia_d = nc.declare_dram_parameter("Ia", [P, P], F16, isOutput=False)
    sdm_d = nc.declare_dram_parameter("Sdm", [P, P], F16, isOutput=False)
    sup_d = nc.declare_dram_parameter("Sup", [P, P], F16, isOutput=False)
    inz_d = nc.declare_dram_parameter("Inz", [P, P], F16, isOutput=False)
    ts_d = nc.declare_dram_parameter("ts", [P, NSNAP * FREE], F16, isOutput=True)
    eden_d = nc.declare_dram_parameter("eden", [P, 2 * K], F32, isOutput=True)

    with tile.TileContext(nc) as tc, ExitStack() as ctx:
        pool = ctx.enter_context(tc.tile_pool(name="st", bufs=1))
        pspool = ctx.enter_context(tc.tile_pool(name="ps", bufs=1, space="PSUM"))

        def T(name, shape=(P, FREE), dt=F16):
            return pool.tile(list(shape), dt, name=name, tag=name)

        img = T("img_t"); p0 = T("p0"); p1 = T("p1")
        Bp = T("Bp"); dneg = T("dneg"); tscr = T("tscr")
        g0 = T("g0"); g1 = T("g1")
        sq0 = T("sq0"); n2 = T("n2"); scr = T("scr")
        r = T("r"); u0 = T("u0"); u1 = T("u1")
        s32 = T("s32", dt=F32); d32 = T("d32", dt=F32); rf = T("rf", dt=F32)
        Ia = T("Ia_t", (P, P)); Sdm = T("Sdm_t", (P, P))
        Sup = T("Sup_t", (P, P)); Inz = T("Inz_t", (P, P))
        eden = T("eden", (P, 2 * K), F32)
        snaps = [T(f"snap{i}") for i in range(NSNAP)]
        psum0 = pspool.tile([P, WT], F32, name="psum0", tag="psum0")
        psum3 = pspool.tile([P, WT], F32, name="psum3", tag="psum3")

        nc.sync.dma_start(img[:], img_d.ap())
        nc.sync.dma_start(Ia[:], ia_d.ap())
        nc.sync.dma_start(Sdm[:], sdm_d.ap())
        nc.sync.dma_start(Sup[:], sup_d.ap())
        nc.sync.dma_start(Inz[:], inz_d.ap())

        nc.vector.memset(p0[:], 0.0)
        nc.vector.memset(p1[:], 0.0)
        nc.vector.memset(g1[:], 0.0)   # col WT-1 must stay 0 (never written in loop)

        def v3(ap):
            return ap.rearrange("p (j w) -> p j w", w=WT)

        for j in range(K):
            t = snaps[j - J_LO] if j >= J_LO else tscr
            p03 = v3(p0[:]); p13 = v3(p1[:]); d3 = v3(dneg[:])
            t3 = v3(t[:]); g03 = v3(g0[:]); g13 = v3(g1[:]); Bp3 = v3(Bp[:])

            # strip-boundary block of (p0 - shiftH p0): psum0 = I@p0_s0 - Sd@p0_s3
            nc.tensor.matmul(psum0[:], Ia[:], p0[:, 0:WT], start=True, stop=False)
            nc.tensor.matmul(psum0[:], Sdm[:], p0[:, 3 * WT:], start=False, stop=True)

            # Bp = p1 - shiftW(p1)  (GPSIMD)
            nc.gpsimd.tensor_copy(Bp3[:, :, 0:1], p13[:, :, 0:1])
            nc.gpsimd.tensor_tensor(Bp3[:, :, 1:WT], p13[:, :, 1:WT],
                                    p13[:, :, 0:WT - 1], ALU.subtract)

            # interior of (p0' - shiftH p0')   [p-tiles hold p/tau]
            nc.vector.tensor_tensor(d3[:, 1:4, :], p03[:, 1:4, :],
                                    p03[:, 0:3, :], ALU.subtract)
            # boundary block from PSUM
            nc.scalar.activation(d3[:, 0, :], psum0[:], ACTF.Copy)
            # dneg' = (p0' - shiftH p0') + (p1' - shiftW p1')   [= dneg/tau]
            nc.vector.tensor_add(dneg[:], dneg[:], Bp[:])

            # Ed_j = sum((tau*dneg')^2) = sum(dneg^2)
            nc.scalar.activation(scr[:], dneg[:], ACTF.Square, scale=float(TAU),
                                 accum_out=eden[:, 2 * j:2 * j + 1])

            # t = img - tau*dneg'
            nc.vector.scalar_tensor_tensor(t[:], dneg[:], float(-TAU), img[:],
                                           ALU.mult, ALU.add)

            # strip-boundary block of g0: psum3 = Su@t_s0 - Iz@t_s3
            nc.tensor.matmul(psum3[:], Sup[:], t[:, 0:WT], start=True, stop=False)
            nc.tensor.matmul(psum3[:], Inz[:], t[:, 3 * WT:], start=False, stop=True)

            # g0 interior; boundary from PSUM
            nc.vector.tensor_tensor(g03[:, 0:3, :], t3[:, 1:4, :],
                                    t3[:, 0:3, :], ALU.subtract)
            nc.scalar.activation(g03[:, 3, :], psum3[:], ACTF.Copy)

            # g1 = shiftW^-1(t) - t  (GPSIMD; col WT-1 stays 0)
            nc.gpsimd.tensor_tensor(g13[:, :, 0:WT - 1], t3[:, :, 1:WT],
                                    t3[:, :, 0:WT - 1], ALU.subtract)

            # n2 = g0^2 + g1^2
            nc.scalar.activation(sq0[:], g0[:], ACTF.Square)
            nc.scalar.activation(n2[:], g1[:], ACTF.Square)
            nc.vector.tensor_add(n2[:], n2[:], sq0[:])

            # norm = sqrt(n2); En_j = sum(norm)
            nc.scalar.activation(s32[:], n2[:], ACTF.Sqrt,
                                 accum_out=eden[:, 2 * j + 1:2 * j + 2])

            # r = 1 / (1 + (tau/weight)*norm)
            nc.vector.tensor_scalar(d32[:], s32[:], float(C_TW), 1.0,
                                    ALU.mult, ALU.add)
            nc.vector.reciprocal_approx_fast(rf[:], d32[:])
            nc.vector.tensor_copy(r[:], rf[:])

            # p' updates (p'_new = (p' - g)*r, plain TT at 2x);
            # p1 first so next iteration's GPSIMD W-shift starts early
            nc.vector.tensor_sub(u1[:], p1[:], g1[:])
            nc.vector.tensor_mul(p1[:], u1[:], r[:])
            nc.vector.tensor_sub(u0[:], p0[:], g0[:])
            nc.vector.tensor_mul(p0[:], u0[:], r[:])

            if j >= J_LO:
                nc.sync.dma_start(ts_d.ap()[:, (j - J_LO) * FREE:(j - J_LO + 1) * FREE],
                                  t[:])

        nc.sync.dma_start(eden_d.ap(), eden[:])

    nc.compile()
    return nc


def _get_nc():
    global _NC
    if _NC is None:
        _NC = _build()
    return _NC


def _host_reference_fallback(img):
    """Exact CPU port of the reference (incl. freeze); only used if the
    device E-sequence fails to locate i* inside [J_LO, K)."""
    out = np.empty_like(img)
    for c in range(img.shape[0]):
        image = img[c].astype(np.float64)
        Hh, Ww = image.shape
        tau = 0.25
        p = np.zeros((2, Hh, Ww))
        o = image.copy()
        E_init = None
        E_prev = None
        for i in range(200):
            d = -p.sum(0)
            d[1:, :] += p[0, :-1, :]
            d[:, 1:] += p[1, :, :-1]
            o = image + d
            gg0 = np.zeros_like(o); gg0[:-1] = o[1:] - o[:-1]
            gg1 = np.zeros_like(o); gg1[:, :-1] = o[:, 1:] - o[:, :-1]
            nrm = np.sqrt(gg0 * gg0 + gg1 * gg1)
            E = ((d * d).sum() + WEIGHT * nrm.sum()) / (Hh * Ww)
            if i == 0:
                E_init = E
            elif abs(E_prev - E) < EPS * E_init:
                break
            E_prev = E
            p = (p - tau * np.stack([gg0, gg1])) / (1.0 + C_TW * nrm[None])
        out[c] = o.astype(np.float32)
    return out


def kernel(img: np.ndarray) -> np.ndarray:
    from concourse.bass_utils import run_bass_kernel_spmd

    assert img.shape == (3, 512, 512) and img.dtype == np.float32
    nc = _get_nc()
    del LAST_RESULTS[:]

    Ia = np.eye(P, dtype=np.float16)
    Sdm = (-np.eye(P, k=1)).astype(np.float16)   # psum0[m] -= p0_s3[m-1]
    Sup = np.eye(P, k=-1, dtype=np.float16)      # psum3[m] += t_s0[m+1]
    Inz = (-np.eye(P)).astype(np.float16)
    Inz[P - 1, P - 1] = 0.0                      # g0 row 511 = 0

    # core -> (channel, col range of its 288-wide slice)
    col_lo = [0, H - WT]     # half 0: cols 0..287; half 1: cols 224..511
    core_map = [(c // 2, c % 2) for c in range(6)] + [(0, 0), (1, 0)]

    in_maps = []
    for c in range(N_CORES):
        ch, half = core_map[c]
        lo = col_lo[half]
        sl = np.ascontiguousarray(img[ch][:, lo:lo + WT]).astype(np.float16)
        in_maps.append({
            "img": sl.reshape(P, FREE),
            "Ia": Ia, "Sdm": Sdm, "Sup": Sup, "Inz": Inz,
        })

    res = run_bass_kernel_spmd(nc, in_maps, list(range(N_CORES)))
    LAST_RESULTS.append(res)
    outs = res.results

    result = np.empty((3, 512, 512), np.float32)
    ok = True
    for ch in range(3):
        # E_j from the pair's summed partials (scale-invariant stopping rule)
        ed = (outs[2 * ch]["eden"].astype(np.float64).sum(0)
              + outs[2 * ch + 1]["eden"].astype(np.float64).sum(0))
        E = ed[0::2] + WEIGHT * ed[1::2]
        istar = None
        for jj in range(1, K):
            if abs(E[jj - 1] - E[jj]) < EPS * E[0]:
                istar = jj
                break
        DIAG[ch] = (istar, [abs(E[jj - 1] - E[jj]) / (EPS * E[0])
                            for jj in range(1, K)])
        if istar is None or istar < J_LO:
            ok = False
            break
        for half in (0, 1):
            t = outs[2 * ch + half]["ts"][:, (istar - J_LO) * FREE:
                                          (istar - J_LO + 1) * FREE]
            t = t.reshape(H, WT).astype(np.float32)
            if half == 0:
                result[ch][:, 0:OWN] = t[:, 0:OWN]
            else:
                result[ch][:, OWN:H] = t[:, WT - OWN:WT]
    if not ok:
        return _host_reference_fallback(img)
    return result


# revision 15
# speedup vs baseline: 3.1331x; 1.0994x over previous
"""TV-Chambolle denoise (weight=0.1, eps=2e-4, n_iter_max=200) on 8 Trainium2
NeuronCores via Bass/Tile — v2.

Strategy vs v1 (1.1ms):
- Unconditional iterations: the reference's early-stop freeze is emulated on
  the HOST. The device runs K=26 plain Chambolle iterations, accumulates the
  per-iteration energy partial sums Ed_j = sum(d^2), En_j = sum(norm) via ACT
  accum_out, and streams the iterate t_j (j >= J_LO) to DRAM. The host finds
  the freeze iteration i* = first j>=1 with |E_{j-1}-E_j| < eps*E_0 and picks
  t_{i*}. (out_final = img + div(p_{i*}) = t computed during step i*.)
  This removes the long serialized per-iteration convergence chain.
- fp16 tiles: 2x DVE throughput on tensor_tensor (2x_1P mode).
- PE computes the strip-boundary (partition-crossing) stencil blocks directly
  into PSUM via paired accumulating matmuls (I@x - Shift@y), ACT copies them
  out — no DVE halo ops.
- 6 useful cores: channel c is W-split across cores 2c (cols 0..287 of 512,
  owns 0..255) and 2c+1 (cols 224..511, owns 256..511). The 32 ghost columns
  make each half's owned region exact for >= 32 iterations with ZERO
  inter-core communication (1 col/iteration dependency horizon). Cores 6,7
  run duplicate work (ignored).

Layout per core: [128, 4*288] fp16 strip layout — partition p holds image
rows 4p..4p+3 of its 288-col slice.
"""
import sys
if '/opt/trn_rl_repo' not in sys.path:
    sys.path.insert(0, '/opt/trn_rl_repo')

import numpy as np

EPS = 2e-4
WEIGHT = 0.1
TAU = 0.25
C_TW = TAU / WEIGHT

P = 128
J = 4
WT = 288          # per-core tile width (cols): 256 owned + 32 ghost
OWN = 256
GHOST = 32
FREE = J * WT
K = 23            # unconditional iterations per launch
J_LO = 16         # stream t_j for j in [J_LO, K)
NSNAP = K - J_LO
N_CORES = 8
H = 512

_NC = None
LAST_RESULTS = []
DIAG = {}


def _build():
    import concourse.bacc as bacc
    import concourse.tile as tile
    import concourse.mybir as mybir
    from contextlib import ExitStack

    F16 = mybir.dt.float16
    F32 = mybir.dt.float32
    ALU = mybir.AluOpType
    ACTF = mybir.ActivationFunctionType

    nc = bacc.Bacc('TRN2', target_bir_lowering=False, debug=False)

    img_d = nc.declare_dram_parameter("img", [P, FREE], F16, isOutput=False)
    sdm_d = nc.declare_dram_parameter("Sdm", [P, P], F16, isOutput=False)
    sup_d = nc.declare_dram_parameter("Sup", [P, P], F16, isOutput=False)
    inz_d = nc.declare_dram_parameter("Inz", [P, P], F16, isOutput=False)
    ts_d = nc.declare_dram_parameter("ts", [P, NSNAP * FREE], F16, isOutput=True)
    eden_d = nc.declare_dram_parameter("eden", [P, 2 * K], F32, isOutput=True)

    with tile.TileContext(nc) as tc, ExitStack() as ctx:
        pool = ctx.enter_context(tc.tile_pool(name="st", bufs=1))
        pspool = ctx.enter_context(tc.tile_pool(name="ps", bufs=1, space="PSUM"))

        def T(name, shape=(P, FREE), dt=F16):
            return pool.tile(list(shape), dt, name=name, tag=name)

        img = T("img_t"); p0 = T("p0"); p1 = T("p1")
        dneg = T("dneg"); tscr = T("tscr")
        g0 = T("g0"); g1 = T("g1")
        sq0 = T("sq0"); n2 = T("n2"); scr = T("scr")
        r = T("r"); u0 = T("u0"); u1 = T("u1")
        s32 = T("s32", dt=F32); d32 = T("d32", dt=F32); rf = T("rf", dt=F32)
        Sdm = T("Sdm_t", (P, P))
        Sup = T("Sup_t", (P, P)); Inz = T("Inz_t", (P, P))
        eden = T("eden", (P, 2 * K), F32)
        snaps = [T(f"snap{i}") for i in range(NSNAP)]
        psum0 = pspool.tile([P, WT], F32, name="psum0", tag="psum0")
        psum3 = pspool.tile([P, WT], F32, name="psum3", tag="psum3")

        nc.sync.dma_start(img[:], img_d.ap())
        nc.sync.dma_start(Sdm[:], sdm_d.ap())
        nc.sync.dma_start(Sup[:], sup_d.ap())
        nc.sync.dma_start(Inz[:], inz_d.ap())

        nc.vector.memset(p0[:], 0.0)
        nc.vector.memset(p1[:], 0.0)
        nc.vector.memset(g1[:], 0.0)   # col WT-1 must stay 0 (never written in loop)

        def v3(ap):
            return ap.rearrange("p (j w) -> p j w", w=WT)

        for j in range(K):
            t = snaps[j - J_LO] if j >= J_LO else tscr
            p03 = v3(p0[:]); p13 = v3(p1[:]); d3 = v3(dneg[:])
            t3 = v3(t[:]); g03 = v3(g0[:]); g13 = v3(g1[:])

            # halo for strip-0 of the H-shift: psum0 = Sd @ p0_s3 (row above)
            nc.tensor.matmul(psum0[:], Sdm[:], p0[:, 3 * WT:], start=True, stop=True)

            # dneg' = (p0' - shiftH p0') + (p1' - shiftW p1')   [= dneg/tau]
            # built in-place: d = p0+p1; d[s1:] -= p0[s0:s2]; d[:,:,1:] -= p1 shifted;
            # d[s0] -= halo (psum0 holds -p0[row-1] via Sdm = -eye(k=1)).
            nc.vector.tensor_add(dneg[:], p0[:], p1[:])
            nc.vector.tensor_tensor(d3[:, 1:4, :], d3[:, 1:4, :],
                                    p03[:, 0:3, :], ALU.subtract)
            nc.vector.tensor_tensor(d3[:, :, 1:WT], d3[:, :, 1:WT],
                                    p13[:, :, 0:WT - 1], ALU.subtract)
            nc.vector.tensor_tensor(d3[:, 0, :], d3[:, 0, :], psum0[:], ALU.add)

            # Ed_j = sum((tau*dneg')^2) = sum(dneg^2)
            nc.scalar.activation(scr[:], dneg[:], ACTF.Square, scale=float(TAU),
                                 accum_out=eden[:, 2 * j:2 * j + 1])

            # t = img - tau*dneg'
            nc.vector.scalar_tensor_tensor(t[:], dneg[:], float(-TAU), img[:],
                                           ALU.mult, ALU.add)

            # strip-boundary block of g0: psum3 = Su@t_s0 - Iz@t_s3
            nc.tensor.matmul(psum3[:], Sup[:], t[:, 0:WT], start=True, stop=False)
            nc.tensor.matmul(psum3[:], Inz[:], t[:, 3 * WT:], start=False, stop=True)

            # g0 interior; boundary from PSUM
            nc.vector.tensor_tensor(g03[:, 0:3, :], t3[:, 1:4, :],
                                    t3[:, 0:3, :], ALU.subtract)
            nc.scalar.activation(g03[:, 3, :], psum3[:], ACTF.Copy)

            # g1 = shiftW^-1(t) - t  (col WT-1 stays 0)
            nc.vector.tensor_tensor(g13[:, :, 0:WT - 1], t3[:, :, 1:WT],
                                    t3[:, :, 0:WT - 1], ALU.subtract)

            # n2 = g0^2 + g1^2
            nc.scalar.activation(sq0[:], g0[:], ACTF.Square)
            nc.scalar.activation(n2[:], g1[:], ACTF.Square)
            nc.vector.tensor_add(n2[:], n2[:], sq0[:])

            # norm = sqrt(n2); En_j = sum(norm)
            nc.scalar.activation(s32[:], n2[:], ACTF.Sqrt,
                                 accum_out=eden[:, 2 * j + 1:2 * j + 2])

            # r = 1 / (1 + (tau/weight)*norm)
            nc.vector.tensor_scalar(d32[:], s32[:], float(C_TW), 1.0,
                                    ALU.mult, ALU.add)
            nc.vector.reciprocal_approx_fast(rf[:], d32[:])
            nc.vector.tensor_copy(r[:], rf[:])

            # p' updates (p'_new = (p' - g)*r, plain TT at 2x);
            # p1 first so next iteration's GPSIMD W-shift starts early
            nc.vector.tensor_sub(u1[:], p1[:], g1[:])
            nc.vector.tensor_mul(p1[:], u1[:], r[:])
            nc.vector.tensor_sub(u0[:], p0[:], g0[:])
            nc.vector.tensor_mul(p0[:], u0[:], r[:])

            if j >= J_LO:
                nc.sync.dma_start(ts_d.ap()[:, (j - J_LO) * FREE:(j - J_LO + 1) * FREE],
                                  t[:])

        nc.sync.dma_start(eden_d.ap(), eden[:])

    nc.compile()
    return nc


def _get_nc():
    global _NC
    if _NC is None:
        _NC = _build()
    return _NC


def _host_reference_fallback(img):
    """Exact CPU port of the reference (incl. freeze); only used if the
    device E-sequence fails to locate i* inside [J_LO, K)."""
    out = np.empty_like(img)
    for c in range(img.shape[0]):
        image = img[c].astype(np.float64)
        Hh, Ww = image.shape
        tau = 0.25
        p = np.zeros((2, Hh, Ww))
        o = image.copy()
        E_init = None
        E_prev = None
        for i in range(200):
            d = -p.sum(0)
            d[1:, :] += p[0, :-1, :]
            d[:, 1:] += p[1, :, :-1]
            o = image + d
            gg0 = np.zeros_like(o); gg0[:-1] = o[1:] - o[:-1]
            gg1 = np.zeros_like(o); gg1[:, :-1] = o[:, 1:] - o[:, :-1]
            nrm = np.sqrt(gg0 * gg0 + gg1 * gg1)
            E = ((d * d).sum() + WEIGHT * nrm.sum()) / (Hh * Ww)
            if i == 0:
                E_init = E
            elif abs(E_prev - E) < EPS * E_init:
                break
            E_prev = E
            p = (p - tau * np.stack([gg0, gg1])) / (1.0 + C_TW * nrm[None])
        out[c] = o.astype(np.float32)
    return out


def kernel(img: np.ndarray) -> np.ndarray:
    from concourse.bass_utils import run_bass_kernel_spmd

    assert img.shape == (3, 512, 512) and img.dtype == np.float32
    nc = _get_nc()
    del LAST_RESULTS[:]

    Sdm = (-np.eye(P, k=1)).astype(np.float16)   # psum0[m] = -p0_s3[m-1]
    Sup = np.eye(P, k=-1, dtype=np.float16)      # psum3[m] += t_s0[m+1]
    Inz = (-np.eye(P)).astype(np.float16)
    Inz[P - 1, P - 1] = 0.0                      # g0 row 511 = 0

    # core -> (channel, col range of its 288-wide slice)
    col_lo = [0, H - WT]     # half 0: cols 0..287; half 1: cols 224..511
    core_map = [(c // 2, c % 2) for c in range(6)] + [(0, 0), (1, 0)]

    in_maps = []
    for c in range(N_CORES):
        ch, half = core_map[c]
        lo = col_lo[half]
        sl = np.ascontiguousarray(img[ch][:, lo:lo + WT]).astype(np.float16)
        in_maps.append({
            "img": sl.reshape(P, FREE),
            "Sdm": Sdm, "Sup": Sup, "Inz": Inz,
        })

    res = run_bass_kernel_spmd(nc, in_maps, list(range(N_CORES)))
    LAST_RESULTS.append(res)
    outs = res.results

    result = np.empty((3, 512, 512), np.float32)
    ok = True
    for ch in range(3):
        # E_j from the pair's summed partials (scale-invariant stopping rule)
        ed = (outs[2 * ch]["eden"].astype(np.float64).sum(0)
              + outs[2 * ch + 1]["eden"].astype(np.float64).sum(0))
        E = ed[0::2] + WEIGHT * ed[1::2]
        istar = None
        for jj in range(1, K):
            if abs(E[jj - 1] - E[jj]) < EPS * E[0]:
                istar = jj
                break
        DIAG[ch] = (istar, [abs(E[jj - 1] - E[jj]) / (EPS * E[0])
                            for jj in range(1, K)])
        if istar is None or istar < J_LO:
            ok = False
            break
        for half in (0, 1):
            t = outs[2 * ch + half]["ts"][:, (istar - J_LO) * FREE:
                                          (istar - J_LO + 1) * FREE]
            t = t.reshape(H, WT).astype(np.float32)
            if half == 0:
                result[ch][:, 0:OWN] = t[:, 0:OWN]
            else:
                result[ch][:, OWN:H] = t[:, WT - OWN:WT]
    if not ok:
        return _host_reference_fallback(img)
    return result


# revision 18
# speedup vs baseline: 3.3401x; 1.0661x over previous
"""TV-Chambolle denoise (weight=0.1, eps=2e-4, n_iter_max=200) on 8 Trainium2
NeuronCores via Bass/Tile — v2.

Strategy vs v1 (1.1ms):
- Unconditional iterations: the reference's early-stop freeze is emulated on
  the HOST. The device runs K=26 plain Chambolle iterations, accumulates the
  per-iteration energy partial sums Ed_j = sum(d^2), En_j = sum(norm) via ACT
  accum_out, and streams the iterate t_j (j >= J_LO) to DRAM. The host finds
  the freeze iteration i* = first j>=1 with |E_{j-1}-E_j| < eps*E_0 and picks
  t_{i*}. (out_final = img + div(p_{i*}) = t computed during step i*.)
  This removes the long serialized per-iteration convergence chain.
- fp16 tiles: 2x DVE throughput on tensor_tensor (2x_1P mode).
- PE computes the strip-boundary (partition-crossing) stencil blocks directly
  into PSUM via paired accumulating matmuls (I@x - Shift@y), ACT copies them
  out — no DVE halo ops.
- 6 useful cores: channel c is W-split across cores 2c (cols 0..287 of 512,
  owns 0..255) and 2c+1 (cols 224..511, owns 256..511). The 32 ghost columns
  make each half's owned region exact for >= 32 iterations with ZERO
  inter-core communication (1 col/iteration dependency horizon). Cores 6,7
  run duplicate work (ignored).

Layout per core: [128, 4*288] fp16 strip layout — partition p holds image
rows 4p..4p+3 of its 288-col slice.
"""
import sys
if '/opt/trn_rl_repo' not in sys.path:
    sys.path.insert(0, '/opt/trn_rl_repo')

import numpy as np

EPS = 2e-4
WEIGHT = 0.1
TAU = 0.25
C_TW = TAU / WEIGHT

P = 128
J = 4
WT = 280          # per-core tile width (cols): 256 owned + 24 ghost
OWN = 256
GHOST = 24
FREE = J * WT
K = 23            # unconditional iterations per launch
J_LO = 16         # stream t_j for j in [J_LO, K)
NSNAP = K - J_LO
N_CORES = 8
H = 512

_NC = None
LAST_RESULTS = []
DIAG = {}


def _build():
    import concourse.bacc as bacc
    import concourse.tile as tile
    import concourse.mybir as mybir
    from contextlib import ExitStack

    F16 = mybir.dt.float16
    F32 = mybir.dt.float32
    ALU = mybir.AluOpType
    ACTF = mybir.ActivationFunctionType

    nc = bacc.Bacc('TRN2', target_bir_lowering=False, debug=False)

    img_d = nc.declare_dram_parameter("img", [P, FREE], F16, isOutput=False)
    sdm_d = nc.declare_dram_parameter("Sdm", [P, P], F16, isOutput=False)
    sup_d = nc.declare_dram_parameter("Sup", [P, P], F16, isOutput=False)
    inz_d = nc.declare_dram_parameter("Inz", [P, P], F16, isOutput=False)
    ts_d = nc.declare_dram_parameter("ts", [P, NSNAP * FREE], F16, isOutput=True)
    eden_d = nc.declare_dram_parameter("eden", [P, 2 * K], F32, isOutput=True)

    with tile.TileContext(nc) as tc, ExitStack() as ctx:
        pool = ctx.enter_context(tc.tile_pool(name="st", bufs=1))
        pspool = ctx.enter_context(tc.tile_pool(name="ps", bufs=1, space="PSUM"))

        def T(name, shape=(P, FREE), dt=F16):
            return pool.tile(list(shape), dt, name=name, tag=name)

        img = T("img_t"); p0 = T("p0"); p1 = T("p1")
        dneg = T("dneg"); tscr = T("tscr")
        g0 = T("g0"); g1 = T("g1")
        sq0 = T("sq0"); n2 = T("n2"); scr = T("scr")
        r = T("r"); u0 = T("u0"); u1 = T("u1")
        s32 = T("s32", dt=F32); d32 = T("d32", dt=F32); rf = T("rf", dt=F32)
        Sdm = T("Sdm_t", (P, P))
        Sup = T("Sup_t", (P, P)); Inz = T("Inz_t", (P, P))
        eden = T("eden", (P, 2 * K), F32)
        snaps = [T(f"snap{i}") for i in range(NSNAP)]
        psum0 = pspool.tile([P, WT], F32, name="psum0", tag="psum0")
        psum3 = pspool.tile([P, WT], F32, name="psum3", tag="psum3")

        # img split into strip-chunks so the load spreads across DMA queues
        for jj in range(4):
            nc.sync.dma_start(img[:, jj * WT:(jj + 1) * WT],
                              img_d.ap()[:, jj * WT:(jj + 1) * WT])
        nc.sync.dma_start(Sdm[:], sdm_d.ap())
        nc.sync.dma_start(Sup[:], sup_d.ap())
        nc.sync.dma_start(Inz[:], inz_d.ap())

        nc.vector.memset(p0[:], 0.0)
        nc.vector.memset(p1[:], 0.0)
        nc.vector.memset(g1[:], 0.0)   # col WT-1 must stay 0 (never written in loop)

        def v3(ap):
            return ap.rearrange("p (j w) -> p j w", w=WT)

        for j in range(K):
            t = snaps[j - J_LO] if j >= J_LO else tscr
            p03 = v3(p0[:]); p13 = v3(p1[:]); d3 = v3(dneg[:])
            t3 = v3(t[:]); g03 = v3(g0[:]); g13 = v3(g1[:])

            # halo for strip-0 of the H-shift: psum0 = Sd @ p0_s3 (row above)
            nc.tensor.matmul(psum0[:], Sdm[:], p0[:, 3 * WT:], start=True, stop=True)

            # dneg' = (p0' - shiftH p0') + (p1' - shiftW p1')   [= dneg/tau]
            # built in-place: d = p0+p1; d[s1:] -= p0[s0:s2]; d[:,:,1:] -= p1 shifted;
            # d[s0] -= halo (psum0 holds -p0[row-1] via Sdm = -eye(k=1)).
            nc.vector.tensor_add(dneg[:], p0[:], p1[:])
            nc.vector.tensor_tensor(d3[:, 1:4, :], d3[:, 1:4, :],
                                    p03[:, 0:3, :], ALU.subtract)
            nc.vector.tensor_tensor(d3[:, :, 1:WT], d3[:, :, 1:WT],
                                    p13[:, :, 0:WT - 1], ALU.subtract)
            nc.vector.tensor_tensor(d3[:, 0, :], d3[:, 0, :], psum0[:], ALU.add)

            # Ed_j = sum((tau*dneg')^2) = sum(dneg^2)
            nc.scalar.activation(scr[:], dneg[:], ACTF.Square, scale=float(TAU),
                                 accum_out=eden[:, 2 * j:2 * j + 1])

            # t = img - tau*dneg'
            nc.vector.scalar_tensor_tensor(t[:], dneg[:], float(-TAU), img[:],
                                           ALU.mult, ALU.add)

            # strip-boundary block of g0: psum3 = Su@t_s0 - Iz@t_s3
            nc.tensor.matmul(psum3[:], Sup[:], t[:, 0:WT], start=True, stop=False)
            nc.tensor.matmul(psum3[:], Inz[:], t[:, 3 * WT:], start=False, stop=True)

            # g0 interior; boundary from PSUM
            nc.vector.tensor_tensor(g03[:, 0:3, :], t3[:, 1:4, :],
                                    t3[:, 0:3, :], ALU.subtract)
            nc.scalar.activation(g03[:, 3, :], psum3[:], ACTF.Copy)

            # g1 = shiftW^-1(t) - t  (col WT-1 stays 0)
            nc.vector.tensor_tensor(g13[:, :, 0:WT - 1], t3[:, :, 1:WT],
                                    t3[:, :, 0:WT - 1], ALU.subtract)

            # n2 = g0^2 + g1^2: sq0 on ACT (off-chain), sq1 on DVE (on-chain)
            nc.scalar.activation(sq0[:], g0[:], ACTF.Square)
            nc.vector.tensor_mul(n2[:], g1[:], g1[:])
            nc.vector.tensor_add(n2[:], n2[:], sq0[:])

            # norm = sqrt(n2); En_j = sum(norm)
            nc.scalar.activation(s32[:], n2[:], ACTF.Sqrt,
                                 accum_out=eden[:, 2 * j + 1:2 * j + 2])

            if j + 1 < K:
                # u's don't need r — they fill the DVE while ACT runs the sqrt
                nc.vector.tensor_sub(u1[:], p1[:], g1[:])
                nc.vector.tensor_sub(u0[:], p0[:], g0[:])
                # r = 1 / (1 + (tau/weight)*norm)
                nc.vector.tensor_scalar(d32[:], s32[:], float(C_TW), 1.0,
                                        ALU.mult, ALU.add)
                nc.vector.reciprocal_approx_fast(rf[:], d32[:])
                nc.vector.tensor_copy(r[:], rf[:])
                # p1 first so the next iteration's d-chain starts earlier
                nc.vector.tensor_mul(p1[:], u1[:], r[:])
                nc.vector.tensor_mul(p0[:], u0[:], r[:])

            if j >= J_LO:
                nc.sync.dma_start(ts_d.ap()[:, (j - J_LO) * FREE:(j - J_LO + 1) * FREE],
                                  t[:])

        nc.sync.dma_start(eden_d.ap(), eden[:])

    nc.compile()
    return nc


def _get_nc():
    global _NC
    if _NC is None:
        _NC = _build()
    return _NC


def _host_reference_fallback(img):
    """Exact CPU port of the reference (incl. freeze); only used if the
    device E-sequence fails to locate i* inside [J_LO, K)."""
    out = np.empty_like(img)
    for c in range(img.shape[0]):
        image = img[c].astype(np.float64)
        Hh, Ww = image.shape
        tau = 0.25
        p = np.zeros((2, Hh, Ww))
        o = image.copy()
        E_init = None
        E_prev = None
        for i in range(200):
            d = -p.sum(0)
            d[1:, :] += p[0, :-1, :]
            d[:, 1:] += p[1, :, :-1]
            o = image + d
            gg0 = np.zeros_like(o); gg0[:-1] = o[1:] - o[:-1]
            gg1 = np.zeros_like(o); gg1[:, :-1] = o[:, 1:] - o[:, :-1]
            nrm = np.sqrt(gg0 * gg0 + gg1 * gg1)
            E = ((d * d).sum() + WEIGHT * nrm.sum()) / (Hh * Ww)
            if i == 0:
                E_init = E
            elif abs(E_prev - E) < EPS * E_init:
                break
            E_prev = E
            p = (p - tau * np.stack([gg0, gg1])) / (1.0 + C_TW * nrm[None])
        out[c] = o.astype(np.float32)
    return out


def kernel(img: np.ndarray) -> np.ndarray:
    from concourse.bass_utils import run_bass_kernel_spmd

    assert img.shape == (3, 512, 512) and img.dtype == np.float32
    nc = _get_nc()
    del LAST_RESULTS[:]

    Sdm = (-np.eye(P, k=1)).astype(np.float16)   # psum0[m] = -p0_s3[m-1]
    Sup = np.eye(P, k=-1, dtype=np.float16)      # psum3[m] += t_s0[m+1]
    Inz = (-np.eye(P)).astype(np.float16)
    Inz[P - 1, P - 1] = 0.0                      # g0 row 511 = 0

    # core -> (channel, col range of its 288-wide slice)
    col_lo = [0, H - WT]     # half 0: cols 0..287; half 1: cols 224..511
    core_map = [(c // 2, c % 2) for c in range(6)] + [(0, 0), (1, 0)]

    in_maps = []
    for c in range(N_CORES):
        ch, half = core_map[c]
        lo = col_lo[half]
        sl = np.ascontiguousarray(img[ch][:, lo:lo + WT]).astype(np.float16)
        in_maps.append({
            "img": sl.reshape(P, FREE),
            "Sdm": Sdm, "Sup": Sup, "Inz": Inz,
        })

    res = run_bass_kernel_spmd(nc, in_maps, list(range(N_CORES)))
    LAST_RESULTS.append(res)
    outs = res.results

    result = np.empty((3, 512, 512), np.float32)
    ok = True
    for ch in range(3):
        # E_j from the pair's summed partials (scale-invariant stopping rule)
        ed = (outs[2 * ch]["eden"].astype(np.float64).sum(0)
              + outs[2 * ch + 1]["eden"].astype(np.float64).sum(0))
        E = ed[0::2] + WEIGHT * ed[1::2]
        istar = None
        for jj in range(1, K):
            if abs(E[jj - 1] - E[jj]) < EPS * E[0]:
                istar = jj
                break
        DIAG[ch] = (istar, [abs(E[jj - 1] - E[jj]) / (EPS * E[0])
                            for jj in range(1, K)])
        if istar is None or istar < J_LO:
            ok = False
            break
        for half in (0, 1):
            t = outs[2 * ch + half]["ts"][:, (istar - J_LO) * FREE:
                                          (istar - J_LO + 1) * FREE]
            t = t.reshape(H, WT).astype(np.float32)
            if half == 0:
                result[ch][:, 0:OWN] = t[:, 0:OWN]
            else:
                result[ch][:, OWN:H] = t[:, WT - OWN:WT]
    if not ok:
        return _host_reference_fallback(img)
    return result


# revision 25
# speedup vs baseline: 3.4780x; 1.0413x over previous
"""TV-Chambolle denoise (weight=0.1, eps=2e-4, n_iter_max=200) on 8 Trainium2
NeuronCores via Bass/Tile — v2.

Strategy vs v1 (1.1ms):
- Unconditional iterations: the reference's early-stop freeze is emulated on
  the HOST. The device runs K=26 plain Chambolle iterations, accumulates the
  per-iteration energy partial sums Ed_j = sum(d^2), En_j = sum(norm) via ACT
  accum_out, and streams the iterate t_j (j >= J_LO) to DRAM. The host finds
  the freeze iteration i* = first j>=1 with |E_{j-1}-E_j| < eps*E_0 and picks
  t_{i*}. (out_final = img + div(p_{i*}) = t computed during step i*.)
  This removes the long serialized per-iteration convergence chain.
- fp16 tiles: 2x DVE throughput on tensor_tensor (2x_1P mode).
- PE computes the strip-boundary (partition-crossing) stencil blocks directly
  into PSUM via paired accumulating matmuls (I@x - Shift@y), ACT copies them
  out — no DVE halo ops.
- 6 useful cores: channel c is W-split across cores 2c (cols 0..287 of 512,
  owns 0..255) and 2c+1 (cols 224..511, owns 256..511). The 32 ghost columns
  make each half's owned region exact for >= 32 iterations with ZERO
  inter-core communication (1 col/iteration dependency horizon). Cores 6,7
  run duplicate work (ignored).

Layout per core: [128, 4*288] fp16 strip layout — partition p holds image
rows 4p..4p+3 of its 288-col slice.
"""
import sys
if '/opt/trn_rl_repo' not in sys.path:
    sys.path.insert(0, '/opt/trn_rl_repo')

import numpy as np

EPS = 2e-4
WEIGHT = 0.1
TAU = 0.25
C_TW = TAU / WEIGHT

P = 128
J = 4
WT = 280          # per-core tile width (cols): 256 owned + 24 ghost
OWN = 256
GHOST = 24
FREE = J * WT
K = 23            # unconditional iterations per launch
J_LO = 16         # stream t_j for j in [J_LO, K)
NSNAP = K - J_LO
N_CORES = 8
H = 512

_NC = None
LAST_RESULTS = []
DIAG = {}


def _build():
    import concourse.bacc as bacc
    import concourse.tile as tile
    import concourse.mybir as mybir
    from contextlib import ExitStack

    F16 = mybir.dt.float16
    F32 = mybir.dt.float32
    ALU = mybir.AluOpType
    ACTF = mybir.ActivationFunctionType

    nc = bacc.Bacc('TRN2', target_bir_lowering=False, debug=False)

    img_d = nc.declare_dram_parameter("img", [P, FREE], F16, isOutput=False)
    ia_d = nc.declare_dram_parameter("Ia", [P, P], F16, isOutput=False)
    sdm_d = nc.declare_dram_parameter("Sdm", [P, P], F16, isOutput=False)
    sup_d = nc.declare_dram_parameter("Sup", [P, P], F16, isOutput=False)
    inz_d = nc.declare_dram_parameter("Inz", [P, P], F16, isOutput=False)
    ts_d = nc.declare_dram_parameter("ts", [P, NSNAP * FREE], F16, isOutput=True)
    eden_d = nc.declare_dram_parameter("eden", [P, 2 * K], F32, isOutput=True)

    with tile.TileContext(nc) as tc, ExitStack() as ctx:
        pool = ctx.enter_context(tc.tile_pool(name="st", bufs=1))
        pspool = ctx.enter_context(tc.tile_pool(name="ps", bufs=1, space="PSUM"))

        def T(name, shape=(P, FREE), dt=F16):
            return pool.tile(list(shape), dt, name=name, tag=name)

        img = T("img_t"); p0 = T("p0"); p1 = T("p1")
        dneg = T("dneg"); tscr = T("tscr"); tscl = T("tscl")
        g0 = T("g0"); g1 = T("g1")
        sq0 = T("sq0"); n2 = T("n2"); scr = T("scr")
        r = T("r"); u0 = T("u0"); u1 = T("u1")
        s32 = T("s32", dt=F32); d32 = T("d32", dt=F32); rf = T("rf", dt=F32)
        Ia = T("Ia_t", (P, P)); Sdm = T("Sdm_t", (P, P))
        Sup = T("Sup_t", (P, P)); Inz = T("Inz_t", (P, P))
        eden = T("eden", (P, 2 * K), F32)
        snaps = [T(f"snap{i}") for i in range(NSNAP)]
        psum0 = pspool.tile([P, WT], F32, name="psum0", tag="psum0")
        psum3 = pspool.tile([P, WT], F32, name="psum3", tag="psum3")

        # img split into strip-chunks so the load spreads across DMA queues
        for jj in range(4):
            nc.sync.dma_start(img[:, jj * WT:(jj + 1) * WT],
                              img_d.ap()[:, jj * WT:(jj + 1) * WT])
        nc.sync.dma_start(Ia[:], ia_d.ap())
        nc.sync.dma_start(Sdm[:], sdm_d.ap())
        nc.sync.dma_start(Sup[:], sup_d.ap())
        nc.sync.dma_start(Inz[:], inz_d.ap())

        nc.vector.memset(g1[:], 0.0)   # col WT-1 must stay 0 (never written in loop)

        def v3(ap):
            return ap.rearrange("p (j w) -> p j w", w=WT)

        # State q = -p/tau (sign flip makes u = q + g and lets iteration 0,
        # where p == 0, collapse to t = img and q_1 = g*r).
        for j in range(K):
            t = snaps[j - J_LO] if j >= J_LO else (img if j == 0 else tscr)
            p03 = v3(p0[:]); p13 = v3(p1[:]); d3 = v3(dneg[:])
            t3 = v3(t[:]); g03 = v3(g0[:]); g13 = v3(g1[:])

            if j > 0:
                # dneg' = -dneg/tau = (q0 - shiftH q0) + (q1 - shiftW q1)
                # strip-0 of the H-part + the q0+q1 base via PE:
                #   psum0 = I@q0_s0 + I@q1_s0 + Sdm@q0_s3   (Sdm = -eye(k=1))
                nc.tensor.matmul(psum0[:], Ia[:], p0[:, 0:WT], start=True, stop=False)
                nc.tensor.matmul(psum0[:], Ia[:], p1[:, 0:WT], start=False, stop=False)
                nc.tensor.matmul(psum0[:], Sdm[:], p0[:, 3 * WT:], start=False, stop=True)
                # strips 1-3 base on DVE; strip 0 from PSUM via ACT
                nc.vector.tensor_add(d3[:, 1:4, :], p03[:, 1:4, :], p13[:, 1:4, :])
                nc.scalar.activation(d3[:, 0, :], psum0[:], ACTF.Copy)
                nc.vector.tensor_tensor(d3[:, 1:4, :], d3[:, 1:4, :],
                                        p03[:, 0:3, :], ALU.subtract)
                nc.vector.tensor_tensor(d3[:, :, 1:WT], d3[:, :, 1:WT],
                                        p13[:, :, 0:WT - 1], ALU.subtract)

                # Ed_j = sum((tau*dneg')^2) = sum(dneg^2)
                nc.scalar.activation(scr[:], dneg[:], ACTF.Square, scale=float(TAU),
                                     accum_out=eden[:, 2 * j:2 * j + 1])

                # t = img + tau*dneg'tile  (dneg'tile = -dneg/tau)
                nc.vector.tensor_scalar(tscl[:], dneg[:], float(TAU), None, ALU.mult)
                nc.vector.tensor_add(t[:], img[:], tscl[:])

            # strip-boundary block of g0: psum3 = Su@t_s0 - Iz@t_s3
            nc.tensor.matmul(psum3[:], Sup[:], t[:, 0:WT], start=True, stop=False)
            nc.tensor.matmul(psum3[:], Inz[:], t[:, 3 * WT:], start=False, stop=True)

            # g0 interior; boundary from PSUM
            nc.vector.tensor_tensor(g03[:, 0:3, :], t3[:, 1:4, :],
                                    t3[:, 0:3, :], ALU.subtract)
            nc.scalar.activation(g03[:, 3, :], psum3[:], ACTF.Copy)

            # g1 = shiftW^-1(t) - t  (col WT-1 stays 0)
            nc.vector.tensor_tensor(g13[:, :, 0:WT - 1], t3[:, :, 1:WT],
                                    t3[:, :, 0:WT - 1], ALU.subtract)

            # n2 = g0^2 + g1^2: sq0 on ACT (off-chain), sq1 on DVE (on-chain)
            nc.scalar.activation(sq0[:], g0[:], ACTF.Square)
            nc.vector.tensor_mul(n2[:], g1[:], g1[:])
            nc.vector.tensor_add(n2[:], n2[:], sq0[:])

            # norm = sqrt(n2); En_j = sum(norm)
            nc.scalar.activation(s32[:], n2[:], ACTF.Sqrt,
                                 accum_out=eden[:, 2 * j + 1:2 * j + 2])

            if j + 1 < K:
                # u's don't need r — they fill the DVE while ACT runs the sqrt
                if j > 0:
                    nc.vector.tensor_add(u1[:], p1[:], g1[:])
                    nc.vector.tensor_add(u0[:], p0[:], g0[:])
                # r = 1 / (1 + (tau/weight)*norm)
                nc.vector.tensor_scalar(d32[:], s32[:], float(C_TW), 1.0,
                                        ALU.mult, ALU.add)
                nc.vector.reciprocal_approx_fast(rf[:], d32[:])
                nc.vector.tensor_copy(r[:], rf[:])
                # p1 first so the next iteration's d-chain starts earlier
                nc.vector.tensor_mul(p1[:], u1[:] if j > 0 else g1[:], r[:])
                nc.vector.tensor_mul(p0[:], u0[:] if j > 0 else g0[:], r[:])

            if j >= J_LO:
                nc.sync.dma_start(ts_d.ap()[:, (j - J_LO) * FREE:(j - J_LO + 1) * FREE],
                                  t[:])

        nc.sync.dma_start(eden_d.ap(), eden[:])

    nc.compile()
    return nc


def _get_nc():
    global _NC
    if _NC is None:
        _NC = _build()
    return _NC


def _host_reference_fallback(img):
    """Exact CPU port of the reference (incl. freeze); only used if the
    device E-sequence fails to locate i* inside [J_LO, K)."""
    out = np.empty_like(img)
    for c in range(img.shape[0]):
        image = img[c].astype(np.float64)
        Hh, Ww = image.shape
        tau = 0.25
        p = np.zeros((2, Hh, Ww))
        o = image.copy()
        E_init = None
        E_prev = None
        for i in range(200):
            d = -p.sum(0)
            d[1:, :] += p[0, :-1, :]
            d[:, 1:] += p[1, :, :-1]
            o = image + d
            gg0 = np.zeros_like(o); gg0[:-1] = o[1:] - o[:-1]
            gg1 = np.zeros_like(o); gg1[:, :-1] = o[:, 1:] - o[:, :-1]
            nrm = np.sqrt(gg0 * gg0 + gg1 * gg1)
            E = ((d * d).sum() + WEIGHT * nrm.sum()) / (Hh * Ww)
            if i == 0:
                E_init = E
            elif abs(E_prev - E) < EPS * E_init:
                break
            E_prev = E
            p = (p - tau * np.stack([gg0, gg1])) / (1.0 + C_TW * nrm[None])
        out[c] = o.astype(np.float32)
    return out


def kernel(img: np.ndarray) -> np.ndarray:
    from concourse.bass_utils import run_bass_kernel_spmd

    assert img.shape == (3, 512, 512) and img.dtype == np.float32
    nc = _get_nc()
    del LAST_RESULTS[:]

    Ia = np.eye(P, dtype=np.float16)
    Sdm = (-np.eye(P, k=1)).astype(np.float16)   # psum0[m] -= q0_s3[m-1]
    Sup = np.eye(P, k=-1, dtype=np.float16)      # psum3[m] += t_s0[m+1]
    Inz = (-np.eye(P)).astype(np.float16)
    Inz[P - 1, P - 1] = 0.0                      # g0 row 511 = 0

    # core -> (channel, col range of its 288-wide slice)
    col_lo = [0, H - WT]     # half 0: cols 0..287; half 1: cols 224..511
    core_map = [(c // 2, c % 2) for c in range(6)] + [(0, 0), (1, 0)]

    in_maps = []
    for c in range(N_CORES):
        ch, half = core_map[c]
        lo = col_lo[half]
        sl = np.ascontiguousarray(img[ch][:, lo:lo + WT]).astype(np.float16)
        in_maps.append({
            "img": sl.reshape(P, FREE),
            "Ia": Ia, "Sdm": Sdm, "Sup": Sup, "Inz": Inz,
        })

    res = run_bass_kernel_spmd(nc, in_maps, list(range(N_CORES)))
    LAST_RESULTS.append(res)
    outs = res.results

    result = np.empty((3, 512, 512), np.float32)
    ok = True
    for ch in range(3):
        # E_j from the pair's summed partials (scale-invariant stopping rule)
        ed = (outs[2 * ch]["eden"].astype(np.float64).sum(0)
              + outs[2 * ch + 1]["eden"].astype(np.float64).sum(0))
        edp = ed[0::2].copy()
        edp[0] = 0.0     # Ed_0 == 0 (p starts at 0); col 0 is never written
        E = edp + WEIGHT * ed[1::2]
        istar = None
        for jj in range(1, K):
            if abs(E[jj - 1] - E[jj]) < EPS * E[0]:
                istar = jj
                break
        DIAG[ch] = (istar, [abs(E[jj - 1] - E[jj]) / (EPS * E[0])
                            for jj in range(1, K)])
        if istar is None or istar < J_LO:
            ok = False
            break
        for half in (0, 1):
            t = outs[2 * ch + half]["ts"][:, (istar - J_LO) * FREE:
                                          (istar - J_LO + 1) * FREE]
            t = t.reshape(H, WT).astype(np.float32)
            if half == 0:
                result[ch][:, 0:OWN] = t[:, 0:OWN]
            else:
                result[ch][:, OWN:H] = t[:, WT - OWN:WT]
    if not ok:
        return _host_reference_fallback(img)
    return result


# revision 27
# speedup vs baseline: 3.6784x; 1.0576x over previous
"""TV-Chambolle denoise (weight=0.1, eps=2e-4, n_iter_max=200) on 8 Trainium2
NeuronCores via Bass/Tile — v2.

Strategy vs v1 (1.1ms):
- Unconditional iterations: the reference's early-stop freeze is emulated on
  the HOST. The device runs K=26 plain Chambolle iterations, accumulates the
  per-iteration energy partial sums Ed_j = sum(d^2), En_j = sum(norm) via ACT
  accum_out, and streams the iterate t_j (j >= J_LO) to DRAM. The host finds
  the freeze iteration i* = first j>=1 with |E_{j-1}-E_j| < eps*E_0 and picks
  t_{i*}. (out_final = img + div(p_{i*}) = t computed during step i*.)
  This removes the long serialized per-iteration convergence chain.
- fp16 tiles: 2x DVE throughput on tensor_tensor (2x_1P mode).
- PE computes the strip-boundary (partition-crossing) stencil blocks directly
  into PSUM via paired accumulating matmuls (I@x - Shift@y), ACT copies them
  out — no DVE halo ops.
- 6 useful cores: channel c is W-split across cores 2c (cols 0..287 of 512,
  owns 0..255) and 2c+1 (cols 224..511, owns 256..511). The 32 ghost columns
  make each half's owned region exact for >= 32 iterations with ZERO
  inter-core communication (1 col/iteration dependency horizon). Cores 6,7
  run duplicate work (ignored).

Layout per core: [128, 4*288] fp16 strip layout — partition p holds image
rows 4p..4p+3 of its 288-col slice.
"""
import sys
if '/opt/trn_rl_repo' not in sys.path:
    sys.path.insert(0, '/opt/trn_rl_repo')

import numpy as np

EPS = 2e-4
WEIGHT = 0.1
TAU = 0.25
C_TW = TAU / WEIGHT

P = 128
J = 4
WT = 280          # per-core tile width (cols): 256 owned + 24 ghost
OWN = 256
GHOST = 24
FREE = J * WT
K = 23            # unconditional iterations per launch
J_LO = 16         # stream t_j for j in [J_LO, K)
NSNAP = K - J_LO
N_CORES = 8
H = 512

_NC = None
LAST_RESULTS = []
DIAG = {}


def _build():
    import concourse.bacc as bacc
    import concourse.tile as tile
    import concourse.mybir as mybir
    from contextlib import ExitStack

    F16 = mybir.dt.float16
    F32 = mybir.dt.float32
    ALU = mybir.AluOpType
    ACTF = mybir.ActivationFunctionType

    nc = bacc.Bacc('TRN2', target_bir_lowering=False, debug=False)

    img_d = nc.declare_dram_parameter("img", [P, FREE], F16, isOutput=False)
    ia_d = nc.declare_dram_parameter("Ia", [P, P], F16, isOutput=False)
    sdm_d = nc.declare_dram_parameter("Sdm", [P, P], F16, isOutput=False)
    sup_d = nc.declare_dram_parameter("Sup", [P, P], F16, isOutput=False)
    inz_d = nc.declare_dram_parameter("Inz", [P, P], F16, isOutput=False)
    ts_d = nc.declare_dram_parameter("ts", [P, NSNAP * FREE], F16, isOutput=True)
    eden_d = nc.declare_dram_parameter("eden", [P, 2 * K], F32, isOutput=True)

    with tile.TileContext(nc) as tc, ExitStack() as ctx:
        pool = ctx.enter_context(tc.tile_pool(name="st", bufs=1))
        pspool = ctx.enter_context(tc.tile_pool(name="ps", bufs=1, space="PSUM"))

        def T(name, shape=(P, FREE), dt=F16):
            return pool.tile(list(shape), dt, name=name, tag=name)

        img = T("img_t"); p0 = T("p0"); p1 = T("p1")
        dneg = T("dneg"); tscr = T("tscr"); tscl = T("tscl")
        g0 = T("g0"); g1 = T("g1")
        sq0 = T("sq0"); n2 = T("n2"); scr = T("scr")
        r = T("r"); u0 = T("u0"); u1 = T("u1")
        s32 = T("s32", dt=F32); d32 = T("d32", dt=F32); rf = T("rf", dt=F32)
        Ia = T("Ia_t", (P, P)); Sdm = T("Sdm_t", (P, P))
        Sup = T("Sup_t", (P, P)); Inz = T("Inz_t", (P, P))
        eden = T("eden", (P, 2 * K), F32)
        snaps = [T(f"snap{i}") for i in range(NSNAP)]
        psum0 = pspool.tile([P, WT], F32, name="psum0", tag="psum0")
        psum3 = pspool.tile([P, WT], F32, name="psum3", tag="psum3")

        # img split into strip-chunks so the load spreads across DMA queues
        for jj in range(4):
            nc.sync.dma_start(img[:, jj * WT:(jj + 1) * WT],
                              img_d.ap()[:, jj * WT:(jj + 1) * WT])
        nc.sync.dma_start(Ia[:], ia_d.ap())
        nc.sync.dma_start(Sdm[:], sdm_d.ap())
        nc.sync.dma_start(Sup[:], sup_d.ap())
        nc.sync.dma_start(Inz[:], inz_d.ap())

        nc.vector.memset(g1[:], 0.0)   # col WT-1 must stay 0 (never written in loop)

        def v3(ap):
            return ap.rearrange("p (j w) -> p j w", w=WT)

        # State q = -p/tau (sign flip makes u = q + g and lets iteration 0,
        # where p == 0, collapse to t = img and q_1 = g*r).
        for j in range(K):
            t = snaps[j - J_LO] if j >= J_LO else (img if j == 0 else tscr)
            p03 = v3(p0[:]); p13 = v3(p1[:]); d3 = v3(dneg[:])
            t3 = v3(t[:]); g03 = v3(g0[:]); g13 = v3(g1[:])

            if j > 0:
                # dneg' = -dneg/tau = (q0 - shiftH q0) + (q1 - shiftW q1)
                # strip-0 of the H-part + the q0+q1 base via PE:
                #   psum0 = I@q1_s0 + I@q0_s0 + Sdm@q0_s3   (Sdm = -eye(k=1))
                # q1 matmul first: p1 is written before p0 at the end of the
                # previous iteration, so the PE chain starts earlier.
                nc.tensor.matmul(psum0[:], Ia[:], p1[:, 0:WT], start=True, stop=False)
                nc.tensor.matmul(psum0[:], Ia[:], p0[:, 0:WT], start=False, stop=False)
                nc.tensor.matmul(psum0[:], Sdm[:], p0[:, 3 * WT:], start=False, stop=True)
                # strips 1-3 base on DVE; strip 0 from PSUM via ACT
                nc.vector.tensor_add(d3[:, 1:4, :], p03[:, 1:4, :], p13[:, 1:4, :])
                nc.scalar.activation(d3[:, 0, :], psum0[:], ACTF.Copy)
                nc.vector.tensor_tensor(d3[:, 1:4, :], d3[:, 1:4, :],
                                        p03[:, 0:3, :], ALU.subtract)
                nc.vector.tensor_tensor(d3[:, :, 1:WT], d3[:, :, 1:WT],
                                        p13[:, :, 0:WT - 1], ALU.subtract)

                # Ed_j = sum((tau*dneg')^2) = sum(dneg^2)
                nc.scalar.activation(scr[:], dneg[:], ACTF.Square, scale=float(TAU),
                                     accum_out=eden[:, 2 * j:2 * j + 1])

                # t = img + tau*dneg'tile  (dneg'tile = -dneg/tau)
                nc.vector.tensor_scalar(tscl[:], dneg[:], float(TAU), None, ALU.mult)
                nc.vector.tensor_add(t[:], img[:], tscl[:])

            # strip-boundary block of g0: psum3 = Su@t_s0 - Iz@t_s3
            nc.tensor.matmul(psum3[:], Sup[:], t[:, 0:WT], start=True, stop=False)
            nc.tensor.matmul(psum3[:], Inz[:], t[:, 3 * WT:], start=False, stop=True)

            # g0 interior; boundary from PSUM
            nc.vector.tensor_tensor(g03[:, 0:3, :], t3[:, 1:4, :],
                                    t3[:, 0:3, :], ALU.subtract)
            nc.scalar.activation(g03[:, 3, :], psum3[:], ACTF.Copy)

            # g1 = shiftW^-1(t) - t  (col WT-1 stays 0)
            nc.vector.tensor_tensor(g13[:, :, 0:WT - 1], t3[:, :, 1:WT],
                                    t3[:, :, 0:WT - 1], ALU.subtract)

            # n2 = g0^2 + g1^2: sq0 on ACT (off-chain), sq1 on DVE (on-chain)
            nc.scalar.activation(sq0[:], g0[:], ACTF.Square)
            nc.vector.tensor_mul(n2[:], g1[:], g1[:])

            HF = FREE // 2
            ha = (slice(None), slice(0, HF))
            hb = (slice(None), slice(HF, FREE))
            # halved r-chain: DVE's half-A work overlaps ACT's half-B sqrt
            nc.vector.tensor_add(n2[ha], n2[ha], sq0[ha])
            nc.vector.tensor_add(n2[hb], n2[hb], sq0[hb])
            nc.scalar.activation(s32[ha], n2[ha], ACTF.Sqrt)
            nc.scalar.activation(s32[hb], n2[hb], ACTF.Sqrt)
            # En_j = sum(norm): separate off-chain op so nothing waits on the
            # accumulator read
            nc.scalar.activation(scr[:], n2[:], ACTF.Sqrt,
                                 accum_out=eden[:, 2 * j + 1:2 * j + 2])

            if j + 1 < K:
                # u's don't need r — they fill the DVE while ACT runs the sqrt
                if j > 0:
                    nc.vector.tensor_add(u1[:], p1[:], g1[:])
                    nc.vector.tensor_add(u0[:], p0[:], g0[:])
                # r = 1 / (1 + (tau/weight)*norm), in pipelined halves
                for h in (ha, hb):
                    nc.vector.tensor_scalar(d32[h], s32[h], float(C_TW), 1.0,
                                            ALU.mult, ALU.add)
                    nc.vector.reciprocal_approx_fast(rf[h], d32[h])
                    nc.vector.tensor_copy(r[h], rf[h])
                # p1 first so the next iteration's d-chain starts earlier
                nc.vector.tensor_mul(p1[:], u1[:] if j > 0 else g1[:], r[:])
                nc.vector.tensor_mul(p0[:], u0[:] if j > 0 else g0[:], r[:])

            if j >= J_LO:
                nc.sync.dma_start(ts_d.ap()[:, (j - J_LO) * FREE:(j - J_LO + 1) * FREE],
                                  t[:])

        nc.sync.dma_start(eden_d.ap(), eden[:])

    nc.compile()
    return nc


def _get_nc():
    global _NC
    if _NC is None:
        _NC = _build()
    return _NC


def _host_reference_fallback(img):
    """Exact CPU port of the reference (incl. freeze); only used if the
    device E-sequence fails to locate i* inside [J_LO, K)."""
    out = np.empty_like(img)
    for c in range(img.shape[0]):
        image = img[c].astype(np.float64)
        Hh, Ww = image.shape
        tau = 0.25
        p = np.zeros((2, Hh, Ww))
        o = image.copy()
        E_init = None
        E_prev = None
        for i in range(200):
            d = -p.sum(0)
            d[1:, :] += p[0, :-1, :]
            d[:, 1:] += p[1, :, :-1]
            o = image + d
            gg0 = np.zeros_like(o); gg0[:-1] = o[1:] - o[:-1]
            gg1 = np.zeros_like(o); gg1[:, :-1] = o[:, 1:] - o[:, :-1]
            nrm = np.sqrt(gg0 * gg0 + gg1 * gg1)
            E = ((d * d).sum() + WEIGHT * nrm.sum()) / (Hh * Ww)
            if i == 0:
                E_init = E
            elif abs(E_prev - E) < EPS * E_init:
                break
            E_prev = E
            p = (p - tau * np.stack([gg0, gg1])) / (1.0 + C_TW * nrm[None])
        out[c] = o.astype(np.float32)
    return out


def kernel(img: np.ndarray) -> np.ndarray:
    from concourse.bass_utils import run_bass_kernel_spmd

    assert img.shape == (3, 512, 512) and img.dtype == np.float32
    nc = _get_nc()
    del LAST_RESULTS[:]

    Ia = np.eye(P, dtype=np.float16)
    Sdm = (-np.eye(P, k=1)).astype(np.float16)   # psum0[m] -= q0_s3[m-1]
    Sup = np.eye(P, k=-1, dtype=np.float16)      # psum3[m] += t_s0[m+1]
    Inz = (-np.eye(P)).astype(np.float16)
    Inz[P - 1, P - 1] = 0.0                      # g0 row 511 = 0

    # core -> (channel, col range of its 288-wide slice)
    col_lo = [0, H - WT]     # half 0: cols 0..287; half 1: cols 224..511
    core_map = [(c // 2, c % 2) for c in range(6)] + [(0, 0), (1, 0)]

    in_maps = []
    for c in range(N_CORES):
        ch, half = core_map[c]
        lo = col_lo[half]
        sl = np.ascontiguousarray(img[ch][:, lo:lo + WT]).astype(np.float16)
        in_maps.append({
            "img": sl.reshape(P, FREE),
            "Ia": Ia, "Sdm": Sdm, "Sup": Sup, "Inz": Inz,
        })

    res = run_bass_kernel_spmd(nc, in_maps, list(range(N_CORES)))
    LAST_RESULTS.append(res)
    outs = res.results

    result = np.empty((3, 512, 512), np.float32)
    ok = True
    for ch in range(3):
        # E_j from the pair's summed partials (scale-invariant stopping rule)
        ed = (outs[2 * ch]["eden"].astype(np.float64).sum(0)
              + outs[2 * ch + 1]["eden"].astype(np.float64).sum(0))
        edp = ed[0::2].copy()
        edp[0] = 0.0     # Ed_0 == 0 (p starts at 0); col 0 is never written
        E = edp + WEIGHT * ed[1::2]
        istar = None
        for jj in range(1, K):
            if abs(E[jj - 1] - E[jj]) < EPS * E[0]:
                istar = jj
                break
        DIAG[ch] = (istar, [abs(E[jj - 1] - E[jj]) / (EPS * E[0])
                            for jj in range(1, K)])
        if istar is None or istar < J_LO:
            ok = False
            break
        for half in (0, 1):
            t = outs[2 * ch + half]["ts"][:, (istar - J_LO) * FREE:
                                          (istar - J_LO + 1) * FREE]
            t = t.reshape(H, WT).astype(np.float32)
            if half == 0:
                result[ch][:, 0:OWN] = t[:, 0:OWN]
            else:
                result[ch][:, OWN:H] = t[:, WT - OWN:WT]
    if not ok:
        return _host_reference_fallback(img)
    return result


# revision 28
# speedup vs baseline: 3.8789x; 1.0545x over previous
"""TV-Chambolle denoise (weight=0.1, eps=2e-4, n_iter_max=200) on 8 Trainium2
NeuronCores via Bass/Tile — v2.

Strategy vs v1 (1.1ms):
- Unconditional iterations: the reference's early-stop freeze is emulated on
  the HOST. The device runs K=26 plain Chambolle iterations, accumulates the
  per-iteration energy partial sums Ed_j = sum(d^2), En_j = sum(norm) via ACT
  accum_out, and streams the iterate t_j (j >= J_LO) to DRAM. The host finds
  the freeze iteration i* = first j>=1 with |E_{j-1}-E_j| < eps*E_0 and picks
  t_{i*}. (out_final = img + div(p_{i*}) = t computed during step i*.)
  This removes the long serialized per-iteration convergence chain.
- fp16 tiles: 2x DVE throughput on tensor_tensor (2x_1P mode).
- PE computes the strip-boundary (partition-crossing) stencil blocks directly
  into PSUM via paired accumulating matmuls (I@x - Shift@y), ACT copies them
  out — no DVE halo ops.
- 6 useful cores: channel c is W-split across cores 2c (cols 0..287 of 512,
  owns 0..255) and 2c+1 (cols 224..511, owns 256..511). The 32 ghost columns
  make each half's owned region exact for >= 32 iterations with ZERO
  inter-core communication (1 col/iteration dependency horizon). Cores 6,7
  run duplicate work (ignored).

Layout per core: [128, 4*288] fp16 strip layout — partition p holds image
rows 4p..4p+3 of its 288-col slice.
"""
import sys
if '/opt/trn_rl_repo' not in sys.path:
    sys.path.insert(0, '/opt/trn_rl_repo')

import numpy as np

EPS = 2e-4
WEIGHT = 0.1
TAU = 0.25
C_TW = TAU / WEIGHT

P = 128
J = 4
WT = 280          # per-core tile width (cols): 256 owned + 24 ghost
OWN = 256
GHOST = 24
FREE = J * WT
K = 23            # unconditional iterations per launch
J_LO = 16         # stream t_j for j in [J_LO, K)
NSNAP = K - J_LO
N_CORES = 8
H = 512

_NC = None
LAST_RESULTS = []
DIAG = {}


def _build():
    import concourse.bacc as bacc
    import concourse.tile as tile
    import concourse.mybir as mybir
    from contextlib import ExitStack

    F16 = mybir.dt.float16
    F32 = mybir.dt.float32
    ALU = mybir.AluOpType
    ACTF = mybir.ActivationFunctionType

    nc = bacc.Bacc('TRN2', target_bir_lowering=False, debug=False)

    img_d = nc.declare_dram_parameter("img", [P, FREE], F16, isOutput=False)
    ia_d = nc.declare_dram_parameter("Ia", [P, P], F16, isOutput=False)
    sdm_d = nc.declare_dram_parameter("Sdm", [P, P], F16, isOutput=False)
    sup_d = nc.declare_dram_parameter("Sup", [P, P], F16, isOutput=False)
    inz_d = nc.declare_dram_parameter("Inz", [P, P], F16, isOutput=False)
    ts_d = nc.declare_dram_parameter("ts", [P, NSNAP * FREE], F16, isOutput=True)
    eden_d = nc.declare_dram_parameter("eden", [P, 2 * K], F32, isOutput=True)

    with tile.TileContext(nc) as tc, ExitStack() as ctx:
        pool = ctx.enter_context(tc.tile_pool(name="st", bufs=1))
        pspool = ctx.enter_context(tc.tile_pool(name="ps", bufs=1, space="PSUM"))

        def T(name, shape=(P, FREE), dt=F16):
            return pool.tile(list(shape), dt, name=name, tag=name)

        img = T("img_t"); p0 = T("p0"); p1 = T("p1")
        dneg = T("dneg"); tscr = T("tscr"); tscl = T("tscl")
        g0 = T("g0"); g1 = T("g1")
        sq0 = T("sq0"); n2 = T("n2"); scr = T("scr")
        r = T("r"); u0 = T("u0"); u1 = T("u1")
        s32 = T("s32", dt=F32); d32 = T("d32", dt=F32); rf = T("rf", dt=F32)
        Ia = T("Ia_t", (P, P)); Sdm = T("Sdm_t", (P, P))
        Sup = T("Sup_t", (P, P)); Inz = T("Inz_t", (P, P))
        eden = T("eden", (P, 2 * K), F32)
        snaps = [T(f"snap{i}") for i in range(NSNAP)]
        psum0 = pspool.tile([P, WT], F32, name="psum0", tag="psum0")
        psum3 = pspool.tile([P, WT], F32, name="psum3", tag="psum3")

        # img split into strip-chunks so the load spreads across DMA queues
        for jj in range(4):
            nc.sync.dma_start(img[:, jj * WT:(jj + 1) * WT],
                              img_d.ap()[:, jj * WT:(jj + 1) * WT])
        nc.sync.dma_start(Ia[:], ia_d.ap())
        nc.sync.dma_start(Sdm[:], sdm_d.ap())
        nc.sync.dma_start(Sup[:], sup_d.ap())
        nc.sync.dma_start(Inz[:], inz_d.ap())

        nc.vector.memset(g1[:], 0.0)   # col WT-1 must stay 0 (never written in loop)

        def v3(ap):
            return ap.rearrange("p (j w) -> p j w", w=WT)

        # State q = -p/tau (sign flip makes u = q + g and lets iteration 0,
        # where p == 0, collapse to t = img and q_1 = g*r).
        for j in range(K):
            t = snaps[j - J_LO] if j >= J_LO else (img if j == 0 else tscr)
            p03 = v3(p0[:]); p13 = v3(p1[:]); d3 = v3(dneg[:])
            t3 = v3(t[:]); g03 = v3(g0[:]); g13 = v3(g1[:])

            if j > 0:
                # dneg' = -dneg/tau = (q0 - shiftH q0) + (q1 - shiftW q1)
                # strip-0 of the H-part + the q0+q1 base via PE:
                #   psum0 = I@q1_s0 + I@q0_s0 + Sdm@q0_s3   (Sdm = -eye(k=1))
                # q1 matmul first: p1 is written before p0 at the end of the
                # previous iteration, so the PE chain starts earlier.
                nc.tensor.matmul(psum0[:], Ia[:], p1[:, 0:WT], start=True, stop=False)
                nc.tensor.matmul(psum0[:], Ia[:], p0[:, 0:WT], start=False, stop=False)
                nc.tensor.matmul(psum0[:], Sdm[:], p0[:, 3 * WT:], start=False, stop=True)
                # strips 1-3 base on DVE; strip 0 from PSUM via ACT
                nc.vector.tensor_add(d3[:, 1:4, :], p03[:, 1:4, :], p13[:, 1:4, :])
                nc.scalar.activation(d3[:, 0, :], psum0[:], ACTF.Copy)
                nc.vector.tensor_tensor(d3[:, 1:4, :], d3[:, 1:4, :],
                                        p03[:, 0:3, :], ALU.subtract)
                nc.vector.tensor_tensor(d3[:, :, 1:WT], d3[:, :, 1:WT],
                                        p13[:, :, 0:WT - 1], ALU.subtract)

                # Ed_j = sum((tau*dneg')^2) = sum(dneg^2)
                nc.scalar.activation(scr[:], dneg[:], ACTF.Square, scale=float(TAU),
                                     accum_out=eden[:, 2 * j:2 * j + 1])

                # t = img + tau*dneg'tile  (dneg'tile = -dneg/tau)
                nc.vector.tensor_scalar(tscl[:], dneg[:], float(TAU), None, ALU.mult)
                nc.vector.tensor_add(t[:], img[:], tscl[:])

            # strip-boundary block of g0: psum3 = Su@t_s0 - Iz@t_s3
            nc.tensor.matmul(psum3[:], Sup[:], t[:, 0:WT], start=True, stop=False)
            nc.tensor.matmul(psum3[:], Inz[:], t[:, 3 * WT:], start=False, stop=True)

            # g0 interior; boundary from PSUM
            nc.vector.tensor_tensor(g03[:, 0:3, :], t3[:, 1:4, :],
                                    t3[:, 0:3, :], ALU.subtract)
            nc.scalar.activation(g03[:, 3, :], psum3[:], ACTF.Copy)

            # g1 = shiftW^-1(t) - t  (col WT-1 stays 0)
            nc.vector.tensor_tensor(g13[:, :, 0:WT - 1], t3[:, :, 1:WT],
                                    t3[:, :, 0:WT - 1], ALU.subtract)

            # n2 = g0^2 + g1^2: sq0 on ACT (off-chain), sq1 on DVE (on-chain)
            nc.scalar.activation(sq0[:], g0[:], ACTF.Square)
            nc.vector.tensor_mul(n2[:], g1[:], g1[:])

            HF = FREE // 2
            ha = (slice(None), slice(0, HF))
            hb = (slice(None), slice(HF, FREE))
            # halved r-chain: DVE's half-A work overlaps ACT's half-B sqrt
            nc.vector.tensor_add(n2[ha], n2[ha], sq0[ha])
            nc.vector.tensor_add(n2[hb], n2[hb], sq0[hb])
            nc.scalar.activation(s32[ha], n2[ha], ACTF.Sqrt)
            nc.scalar.activation(s32[hb], n2[hb], ACTF.Sqrt)
            # En_j = sum(norm): separate off-chain op so nothing waits on the
            # accumulator read
            nc.scalar.activation(scr[:], n2[:], ACTF.Sqrt,
                                 accum_out=eden[:, 2 * j + 1:2 * j + 2])

            if j + 1 < K:
                # u's don't need r — they fill the DVE while ACT runs the sqrt
                if j > 0:
                    nc.vector.tensor_add(u1[:], p1[:], g1[:])
                    nc.vector.tensor_add(u0[:], p0[:], g0[:])
                # r = 1 / (1 + (tau/weight)*norm), in pipelined halves.
                # The recip writes fp16 directly (the fp32 bit-trick is on the
                # INPUT; the output conversion is the normal DVE write path),
                # which removes the cast op.
                from concourse.dve_ops import (RECIP_APPROX_FAST_CONSTS,
                                               RECIPROCAL_APPROX_FAST)
                c = RECIP_APPROX_FAST_CONSTS
                for h in (ha, hb):
                    nc.vector.tensor_scalar(d32[h], s32[h], float(C_TW), 1.0,
                                            ALU.mult, ALU.add)
                    nc.vector._custom_dve(RECIPROCAL_APPROX_FAST, out=r[h],
                                          in0=d32[h], s0=c["s0"], s1=c["s1"],
                                          imm2=c["imm2"])
                # p1 first so the next iteration's d-chain starts earlier
                nc.vector.tensor_mul(p1[:], u1[:] if j > 0 else g1[:], r[:])
                nc.vector.tensor_mul(p0[:], u0[:] if j > 0 else g0[:], r[:])

            if j >= J_LO:
                nc.sync.dma_start(ts_d.ap()[:, (j - J_LO) * FREE:(j - J_LO + 1) * FREE],
                                  t[:])

        nc.sync.dma_start(eden_d.ap(), eden[:])

    nc.compile()
    return nc


def _get_nc():
    global _NC
    if _NC is None:
        _NC = _build()
    return _NC


def _host_reference_fallback(img):
    """Exact CPU port of the reference (incl. freeze); only used if the
    device E-sequence fails to locate i* inside [J_LO, K)."""
    out = np.empty_like(img)
    for c in range(img.shape[0]):
        image = img[c].astype(np.float64)
        Hh, Ww = image.shape
        tau = 0.25
        p = np.zeros((2, Hh, Ww))
        o = image.copy()
        E_init = None
        E_prev = None
        for i in range(200):
            d = -p.sum(0)
            d[1:, :] += p[0, :-1, :]
            d[:, 1:] += p[1, :, :-1]
            o = image + d
            gg0 = np.zeros_like(o); gg0[:-1] = o[1:] - o[:-1]
            gg1 = np.zeros_like(o); gg1[:, :-1] = o[:, 1:] - o[:, :-1]
            nrm = np.sqrt(gg0 * gg0 + gg1 * gg1)
            E = ((d * d).sum() + WEIGHT * nrm.sum()) / (Hh * Ww)
            if i == 0:
                E_init = E
            elif abs(E_prev - E) < EPS * E_init:
                break
            E_prev = E
            p = (p - tau * np.stack([gg0, gg1])) / (1.0 + C_TW * nrm[None])
        out[c] = o.astype(np.float32)
    return out


def kernel(img: np.ndarray) -> np.ndarray:
    from concourse.bass_utils import run_bass_kernel_spmd

    assert img.shape == (3, 512, 512) and img.dtype == np.float32
    nc = _get_nc()
    del LAST_RESULTS[:]

    Ia = np.eye(P, dtype=np.float16)
    Sdm = (-np.eye(P, k=1)).astype(np.float16)   # psum0[m] -= q0_s3[m-1]
    Sup = np.eye(P, k=-1, dtype=np.float16)      # psum3[m] += t_s0[m+1]
    Inz = (-np.eye(P)).astype(np.float16)
    Inz[P - 1, P - 1] = 0.0                      # g0 row 511 = 0

    # core -> (channel, col range of its 288-wide slice)
    col_lo = [0, H - WT]     # half 0: cols 0..287; half 1: cols 224..511
    core_map = [(c // 2, c % 2) for c in range(6)] + [(0, 0), (1, 0)]

    in_maps = []
    for c in range(N_CORES):
        ch, half = core_map[c]
        lo = col_lo[half]
        sl = np.ascontiguousarray(img[ch][:, lo:lo + WT]).astype(np.float16)
        in_maps.append({
            "img": sl.reshape(P, FREE),
            "Ia": Ia, "Sdm": Sdm, "Sup": Sup, "Inz": Inz,
        })

    res = run_bass_kernel_spmd(nc, in_maps, list(range(N_CORES)))
    LAST_RESULTS.append(res)
    outs = res.results

    result = np.empty((3, 512, 512), np.float32)
    ok = True
    for ch in range(3):
        # E_j from the pair's summed partials (scale-invariant stopping rule)
        ed = (outs[2 * ch]["eden"].astype(np.float64).sum(0)
              + outs[2 * ch + 1]["eden"].astype(np.float64).sum(0))
        edp = ed[0::2].copy()
        edp[0] = 0.0     # Ed_0 == 0 (p starts at 0); col 0 is never written
        E = edp + WEIGHT * ed[1::2]
        istar = None
        for jj in range(1, K):
            if abs(E[jj - 1] - E[jj]) < EPS * E[0]:
                istar = jj
                break
        DIAG[ch] = (istar, [abs(E[jj - 1] - E[jj]) / (EPS * E[0])
                            for jj in range(1, K)])
        if istar is None or istar < J_LO:
            ok = False
            break
        for half in (0, 1):
            t = outs[2 * ch + half]["ts"][:, (istar - J_LO) * FREE:
                                          (istar - J_LO + 1) * FREE]
            t = t.reshape(H, WT).astype(np.float32)
            if half == 0:
                result[ch][:, 0:OWN] = t[:, 0:OWN]
            else:
                result[ch][:, OWN:H] = t[:, WT - OWN:WT]
    if not ok:
        return _host_reference_fallback(img)
    return result


# revision 30
# speedup vs baseline: 3.8909x; 1.0031x over previous
"""TV-Chambolle denoise (weight=0.1, eps=2e-4, n_iter_max=200) on 8 Trainium2
NeuronCores via Bass/Tile — v2.

Strategy vs v1 (1.1ms):
- Unconditional iterations: the reference's early-stop freeze is emulated on
  the HOST. The device runs K=26 plain Chambolle iterations, accumulates the
  per-iteration energy partial sums Ed_j = sum(d^2), En_j = sum(norm) via ACT
  accum_out, and streams the iterate t_j (j >= J_LO) to DRAM. The host finds
  the freeze iteration i* = first j>=1 with |E_{j-1}-E_j| < eps*E_0 and picks
  t_{i*}. (out_final = img + div(p_{i*}) = t computed during step i*.)
  This removes the long serialized per-iteration convergence chain.
- fp16 tiles: 2x DVE throughput on tensor_tensor (2x_1P mode).
- PE computes the strip-boundary (partition-crossing) stencil blocks directly
  into PSUM via paired accumulating matmuls (I@x - Shift@y), ACT copies them
  out — no DVE halo ops.
- 6 useful cores: channel c is W-split across cores 2c (cols 0..287 of 512,
  owns 0..255) and 2c+1 (cols 224..511, owns 256..511). The 32 ghost columns
  make each half's owned region exact for >= 32 iterations with ZERO
  inter-core communication (1 col/iteration dependency horizon). Cores 6,7
  run duplicate work (ignored).

Layout per core: [128, 4*288] fp16 strip layout — partition p holds image
rows 4p..4p+3 of its 288-col slice.
"""
import sys
if '/opt/trn_rl_repo' not in sys.path:
    sys.path.insert(0, '/opt/trn_rl_repo')

import numpy as np

EPS = 2e-4
WEIGHT = 0.1
TAU = 0.25
C_TW = TAU / WEIGHT

P = 128
J = 4
WT = 280          # per-core tile width (cols): 256 owned + 24 ghost
OWN = 256
GHOST = 24
FREE = J * WT
K = 23            # unconditional iterations per launch
J_LO = 16         # stream t_j for j in [J_LO, K)
NSNAP = K - J_LO
N_CORES = 8
H = 512

_NC = None
LAST_RESULTS = []
DIAG = {}


def _build():
    import concourse.bacc as bacc
    import concourse.tile as tile
    import concourse.mybir as mybir
    from contextlib import ExitStack

    F16 = mybir.dt.float16
    F32 = mybir.dt.float32
    ALU = mybir.AluOpType
    ACTF = mybir.ActivationFunctionType

    nc = bacc.Bacc('TRN2', target_bir_lowering=False, debug=False)

    img_d = nc.declare_dram_parameter("img", [P, FREE], F16, isOutput=False)
    ia_d = nc.declare_dram_parameter("Ia", [P, P], F16, isOutput=False)
    sdm_d = nc.declare_dram_parameter("Sdm", [P, P], F16, isOutput=False)
    sup_d = nc.declare_dram_parameter("Sup", [P, P], F16, isOutput=False)
    inz_d = nc.declare_dram_parameter("Inz", [P, P], F16, isOutput=False)
    ts_d = nc.declare_dram_parameter("ts", [P, NSNAP * FREE], F16, isOutput=True)
    eden_d = nc.declare_dram_parameter("eden", [P, 2 * K], F32, isOutput=True)

    with tile.TileContext(nc) as tc, ExitStack() as ctx:
        pool = ctx.enter_context(tc.tile_pool(name="st", bufs=1))
        pspool = ctx.enter_context(tc.tile_pool(name="ps", bufs=1, space="PSUM"))

        def T(name, shape=(P, FREE), dt=F16):
            return pool.tile(list(shape), dt, name=name, tag=name)

        img = T("img_t"); p0 = T("p0"); p1 = T("p1")
        dneg = T("dneg"); tscr = T("tscr"); tscl = T("tscl")
        g0 = T("g0"); g1 = T("g1")
        sq0 = T("sq0"); n2 = T("n2"); scr = T("scr")
        r = T("r"); u0 = T("u0"); u1 = T("u1")
        s32 = T("s32", dt=F32); d32 = T("d32", dt=F32); rf = T("rf", dt=F32)
        Ia = T("Ia_t", (P, P)); Sdm = T("Sdm_t", (P, P))
        Sup = T("Sup_t", (P, P)); Inz = T("Inz_t", (P, P))
        eden = T("eden", (P, 2 * K), F32)
        snaps = [T(f"snap{i}") for i in range(NSNAP)]
        psum0 = pspool.tile([P, WT], F32, name="psum0", tag="psum0")
        psum3 = pspool.tile([P, WT], F32, name="psum3", tag="psum3")

        # img split into strip-chunks so the load spreads across DMA queues
        for jj in range(4):
            nc.sync.dma_start(img[:, jj * WT:(jj + 1) * WT],
                              img_d.ap()[:, jj * WT:(jj + 1) * WT])
        nc.sync.dma_start(Ia[:], ia_d.ap())
        nc.sync.dma_start(Sdm[:], sdm_d.ap())
        nc.sync.dma_start(Sup[:], sup_d.ap())
        nc.sync.dma_start(Inz[:], inz_d.ap())

        nc.vector.memset(g1[:], 0.0)   # col WT-1 must stay 0 (never written in loop)

        def v3(ap):
            return ap.rearrange("p (j w) -> p j w", w=WT)

        # State q = -p/tau (sign flip makes u = q + g and lets iteration 0,
        # where p == 0, collapse to t = img and q_1 = g*r).
        for j in range(K):
            t = snaps[j - J_LO] if j >= J_LO else (img if j == 0 else tscr)
            p03 = v3(p0[:]); p13 = v3(p1[:]); d3 = v3(dneg[:])
            t3 = v3(t[:]); g03 = v3(g0[:]); g13 = v3(g1[:])

            if j > 0:
                # dneg' = -dneg/tau = (q0 - shiftH q0) + (q1 - shiftW q1)
                # strip-0 of the H-part + the q0+q1 base via PE:
                #   psum0 = I@q1_s0 + I@q0_s0 + Sdm@q0_s3   (Sdm = -eye(k=1))
                # q1 matmul first: p1 is written before p0 at the end of the
                # previous iteration, so the PE chain starts earlier.
                nc.tensor.matmul(psum0[:], Ia[:], p1[:, 0:WT], start=True, stop=False)
                nc.tensor.matmul(psum0[:], Ia[:], p0[:, 0:WT], start=False, stop=False)
                nc.tensor.matmul(psum0[:], Sdm[:], p0[:, 3 * WT:], start=False, stop=True)
                # strips 1-3 base on DVE; strip 0 from PSUM via ACT
                nc.vector.tensor_add(d3[:, 1:4, :], p03[:, 1:4, :], p13[:, 1:4, :])
                nc.scalar.activation(d3[:, 0, :], psum0[:], ACTF.Copy)
                nc.vector.tensor_tensor(d3[:, 1:4, :], d3[:, 1:4, :],
                                        p03[:, 0:3, :], ALU.subtract)
                nc.vector.tensor_tensor(d3[:, :, 1:WT], d3[:, :, 1:WT],
                                        p13[:, :, 0:WT - 1], ALU.subtract)

                # Ed_j = sum((tau*dneg')^2) = sum(dneg^2)
                nc.scalar.activation(scr[:], dneg[:], ACTF.Square, scale=float(TAU),
                                     accum_out=eden[:, 2 * j:2 * j + 1])

                # t = img + tau*dneg'tile  (dneg'tile = -dneg/tau)
                nc.vector.tensor_scalar(tscl[:], dneg[:], float(TAU), None, ALU.mult)
                nc.vector.tensor_add(t[:], img[:], tscl[:])

            # strip-boundary block of g0: psum3 = Su@t_s0 - Iz@t_s3
            nc.tensor.matmul(psum3[:], Sup[:], t[:, 0:WT], start=True, stop=False)
            nc.tensor.matmul(psum3[:], Inz[:], t[:, 3 * WT:], start=False, stop=True)

            # g0 interior; boundary from PSUM
            nc.vector.tensor_tensor(g03[:, 0:3, :], t3[:, 1:4, :],
                                    t3[:, 0:3, :], ALU.subtract)
            nc.scalar.activation(g03[:, 3, :], psum3[:], ACTF.Copy)

            # g1 = shiftW^-1(t) - t  (col WT-1 stays 0)
            nc.vector.tensor_tensor(g13[:, :, 0:WT - 1], t3[:, :, 1:WT],
                                    t3[:, :, 0:WT - 1], ALU.subtract)

            # n2 = g0^2 + g1^2: sq0 on ACT (off-chain), sq1 on DVE (on-chain)
            nc.scalar.activation(sq0[:], g0[:], ACTF.Square)
            nc.vector.tensor_mul(n2[:], g1[:], g1[:])

            HF = FREE // 2
            ha = (slice(None), slice(0, HF))
            hb = (slice(None), slice(HF, FREE))
            # halved r-chain: DVE's half-A work overlaps ACT's half-B sqrt
            nc.vector.tensor_add(n2[ha], n2[ha], sq0[ha])
            nc.vector.tensor_add(n2[hb], n2[hb], sq0[hb])
            if j + 1 < K:
                nc.scalar.activation(s32[ha], n2[ha], ACTF.Sqrt)
                nc.scalar.activation(s32[hb], n2[hb], ACTF.Sqrt)
            # En_j = sum(norm): separate off-chain op so nothing waits on the
            # accumulator read
            nc.scalar.activation(scr[:], n2[:], ACTF.Sqrt,
                                 accum_out=eden[:, 2 * j + 1:2 * j + 2])

            if j + 1 < K:
                # u's don't need r — they fill the DVE while ACT runs the sqrt
                if j > 0:
                    nc.vector.tensor_add(u1[:], p1[:], g1[:])
                    nc.vector.tensor_add(u0[:], p0[:], g0[:])
                # r = 1 / (1 + (tau/weight)*norm), in pipelined halves.
                # The recip writes fp16 directly (the fp32 bit-trick is on the
                # INPUT; the output conversion is the normal DVE write path),
                # which removes the cast op.
                from concourse.dve_ops import (RECIP_APPROX_FAST_CONSTS,
                                               RECIPROCAL_APPROX_FAST)
                c = RECIP_APPROX_FAST_CONSTS
                for h in (ha, hb):
                    nc.vector.tensor_scalar(d32[h], s32[h], float(C_TW), 1.0,
                                            ALU.mult, ALU.add)
                    nc.vector._custom_dve(RECIPROCAL_APPROX_FAST, out=r[h],
                                          in0=d32[h], s0=c["s0"], s1=c["s1"],
                                          imm2=c["imm2"])
                # p1 first so the next iteration's d-chain starts earlier
                nc.vector.tensor_mul(p1[:], u1[:] if j > 0 else g1[:], r[:])
                nc.vector.tensor_mul(p0[:], u0[:] if j > 0 else g0[:], r[:])

            if j >= J_LO:
                # 4 chunks land on different HW DMA queues -> ~4x faster drain,
                # which matters for the last snapshot (nothing hides its tail)
                base = (j - J_LO) * FREE
                for jj in range(4):
                    nc.sync.dma_start(
                        ts_d.ap()[:, base + jj * WT:base + (jj + 1) * WT],
                        t[:, jj * WT:(jj + 1) * WT])

        nc.sync.dma_start(eden_d.ap(), eden[:])

    nc.compile()
    return nc


def _get_nc():
    global _NC
    if _NC is None:
        _NC = _build()
    return _NC


def _host_reference_fallback(img):
    """Exact CPU port of the reference (incl. freeze); only used if the
    device E-sequence fails to locate i* inside [J_LO, K)."""
    out = np.empty_like(img)
    for c in range(img.shape[0]):
        image = img[c].astype(np.float64)
        Hh, Ww = image.shape
        tau = 0.25
        p = np.zeros((2, Hh, Ww))
        o = image.copy()
        E_init = None
        E_prev = None
        for i in range(200):
            d = -p.sum(0)
            d[1:, :] += p[0, :-1, :]
            d[:, 1:] += p[1, :, :-1]
            o = image + d
            gg0 = np.zeros_like(o); gg0[:-1] = o[1:] - o[:-1]
            gg1 = np.zeros_like(o); gg1[:, :-1] = o[:, 1:] - o[:, :-1]
            nrm = np.sqrt(gg0 * gg0 + gg1 * gg1)
            E = ((d * d).sum() + WEIGHT * nrm.sum()) / (Hh * Ww)
            if i == 0:
                E_init = E
            elif abs(E_prev - E) < EPS * E_init:
                break
            E_prev = E
            p = (p - tau * np.stack([gg0, gg1])) / (1.0 + C_TW * nrm[None])
        out[c] = o.astype(np.float32)
    return out


def kernel(img: np.ndarray) -> np.ndarray:
    from concourse.bass_utils import run_bass_kernel_spmd

    assert img.shape == (3, 512, 512) and img.dtype == np.float32
    nc = _get_nc()
    del LAST_RESULTS[:]

    Ia = np.eye(P, dtype=np.float16)
    Sdm = (-np.eye(P, k=1)).astype(np.float16)   # psum0[m] -= q0_s3[m-1]
    Sup = np.eye(P, k=-1, dtype=np.float16)      # psum3[m] += t_s0[m+1]
    Inz = (-np.eye(P)).astype(np.float16)
    Inz[P - 1, P - 1] = 0.0                      # g0 row 511 = 0

    # core -> (channel, col range of its 288-wide slice)
    col_lo = [0, H - WT]     # half 0: cols 0..287; half 1: cols 224..511
    core_map = [(c // 2, c % 2) for c in range(6)] + [(0, 0), (1, 0)]

    in_maps = []
    for c in range(N_CORES):
        ch, half = core_map[c]
        lo = col_lo[half]
        sl = np.ascontiguousarray(img[ch][:, lo:lo + WT]).astype(np.float16)
        in_maps.append({
            "img": sl.reshape(P, FREE),
            "Ia": Ia, "Sdm": Sdm, "Sup": Sup, "Inz": Inz,
        })

    res = run_bass_kernel_spmd(nc, in_maps, list(range(N_CORES)))
    LAST_RESULTS.append(res)
    outs = res.results

    result = np.empty((3, 512, 512), np.float32)
    ok = True
    for ch in range(3):
        # E_j from the pair's summed partials (scale-invariant stopping rule)
        ed = (outs[2 * ch]["eden"].astype(np.float64).sum(0)
              + outs[2 * ch + 1]["eden"].astype(np.float64).sum(0))
        edp = ed[0::2].copy()
        edp[0] = 0.0     # Ed_0 == 0 (p starts at 0); col 0 is never written
        E = edp + WEIGHT * ed[1::2]
        istar = None
        for jj in range(1, K):
            if abs(E[jj - 1] - E[jj]) < EPS * E[0]:
                istar = jj
                break
        DIAG[ch] = (istar, [abs(E[jj - 1] - E[jj]) / (EPS * E[0])
                            for jj in range(1, K)])
        if istar is None or istar < J_LO:
            ok = False
            break
        for half in (0, 1):
            t = outs[2 * ch + half]["ts"][:, (istar - J_LO) * FREE:
                                          (istar - J_LO + 1) * FREE]
            t = t.reshape(H, WT).astype(np.float32)
            if half == 0:
                result[ch][:, 0:OWN] = t[:, 0:OWN]
            else:
                result[ch][:, OWN:H] = t[:, WT - OWN:WT]
    if not ok:
        return _host_reference_fallback(img)
    return result


# revision 32
# speedup vs baseline: 3.8941x; 1.0008x over previous
"""TV-Chambolle denoise (weight=0.1, eps=2e-4, n_iter_max=200) on 8 Trainium2
NeuronCores via Bass/Tile — v2.

Strategy vs v1 (1.1ms):
- Unconditional iterations: the reference's early-stop freeze is emulated on
  the HOST. The device runs K=26 plain Chambolle iterations, accumulates the
  per-iteration energy partial sums Ed_j = sum(d^2), En_j = sum(norm) via ACT
  accum_out, and streams the iterate t_j (j >= J_LO) to DRAM. The host finds
  the freeze iteration i* = first j>=1 with |E_{j-1}-E_j| < eps*E_0 and picks
  t_{i*}. (out_final = img + div(p_{i*}) = t computed during step i*.)
  This removes the long serialized per-iteration convergence chain.
- fp16 tiles: 2x DVE throughput on tensor_tensor (2x_1P mode).
- PE computes the strip-boundary (partition-crossing) stencil blocks directly
  into PSUM via paired accumulating matmuls (I@x - Shift@y), ACT copies them
  out — no DVE halo ops.
- 6 useful cores: channel c is W-split across cores 2c (cols 0..287 of 512,
  owns 0..255) and 2c+1 (cols 224..511, owns 256..511). The 32 ghost columns
  make each half's owned region exact for >= 32 iterations with ZERO
  inter-core communication (1 col/iteration dependency horizon). Cores 6,7
  run duplicate work (ignored).

Layout per core: [128, 4*288] fp16 strip layout — partition p holds image
rows 4p..4p+3 of its 288-col slice.
"""
import sys
if '/opt/trn_rl_repo' not in sys.path:
    sys.path.insert(0, '/opt/trn_rl_repo')

import numpy as np

EPS = 2e-4
WEIGHT = 0.1
TAU = 0.25
C_TW = TAU / WEIGHT

P = 128
J = 4
WT = 280          # per-core tile width (cols): 256 owned + 24 ghost
OWN = 256
GHOST = 24
FREE = J * WT
K = 23            # unconditional iterations per launch
J_LO = 16         # stream t_j for j in [J_LO, K)
NSNAP = K - J_LO
N_CORES = 8
H = 512

_NC = None
LAST_RESULTS = []
DIAG = {}


def _build():
    import concourse.bacc as bacc
    import concourse.tile as tile
    import concourse.mybir as mybir
    from contextlib import ExitStack

    F16 = mybir.dt.float16
    F32 = mybir.dt.float32
    ALU = mybir.AluOpType
    ACTF = mybir.ActivationFunctionType

    nc = bacc.Bacc('TRN2', target_bir_lowering=False, debug=False)

    img_d = nc.declare_dram_parameter("img", [P, FREE], F16, isOutput=False)
    ia_d = nc.declare_dram_parameter("Ia", [P, P], F16, isOutput=False)
    sdm_d = nc.declare_dram_parameter("Sdm", [P, P], F16, isOutput=False)
    sup_d = nc.declare_dram_parameter("Sup", [P, P], F16, isOutput=False)
    inz_d = nc.declare_dram_parameter("Inz", [P, P], F16, isOutput=False)
    ts_d = nc.declare_dram_parameter("ts", [P, NSNAP * FREE], F16, isOutput=True)
    eden_d = nc.declare_dram_parameter("eden", [P, 2 * K], F32, isOutput=True)

    with tile.TileContext(nc) as tc, ExitStack() as ctx:
        pool = ctx.enter_context(tc.tile_pool(name="st", bufs=1))
        pspool = ctx.enter_context(tc.tile_pool(name="ps", bufs=1, space="PSUM"))

        def T(name, shape=(P, FREE), dt=F16):
            return pool.tile(list(shape), dt, name=name, tag=name)

        img = T("img_t"); p0 = T("p0"); p1 = T("p1")
        dneg = T("dneg"); tscr = T("tscr"); tscl = T("tscl")
        g0 = T("g0"); g1 = T("g1")
        sq0 = T("sq0"); n2a = T("n2a"); n2b = T("n2b"); scr = T("scr")
        r = T("r"); u0 = T("u0"); u1 = T("u1")
        s32 = T("s32", dt=F32); d32 = T("d32", dt=F32); rf = T("rf", dt=F32)
        Ia = T("Ia_t", (P, P)); Sdm = T("Sdm_t", (P, P))
        Sup = T("Sup_t", (P, P)); Inz = T("Inz_t", (P, P))
        eden = T("eden", (P, 2 * K), F32)
        snaps = [T(f"snap{i}") for i in range(NSNAP)]
        psum0 = pspool.tile([P, WT], F32, name="psum0", tag="psum0")
        psum3 = pspool.tile([P, WT], F32, name="psum3", tag="psum3")

        # img split into strip-chunks so the load spreads across DMA queues
        for jj in range(4):
            nc.sync.dma_start(img[:, jj * WT:(jj + 1) * WT],
                              img_d.ap()[:, jj * WT:(jj + 1) * WT])
        nc.sync.dma_start(Ia[:], ia_d.ap())
        nc.sync.dma_start(Sdm[:], sdm_d.ap())
        nc.sync.dma_start(Sup[:], sup_d.ap())
        nc.sync.dma_start(Inz[:], inz_d.ap())

        nc.vector.memset(g1[:], 0.0)   # col WT-1 must stay 0 (never written in loop)

        def v3(ap):
            return ap.rearrange("p (j w) -> p j w", w=WT)

        # State q = -p/tau (sign flip makes u = q + g and lets iteration 0,
        # where p == 0, collapse to t = img and q_1 = g*r).
        for j in range(K):
            t = snaps[j - J_LO] if j >= J_LO else (img if j == 0 else tscr)
            p03 = v3(p0[:]); p13 = v3(p1[:]); d3 = v3(dneg[:])
            t3 = v3(t[:]); g03 = v3(g0[:]); g13 = v3(g1[:])

            if j > 0:
                # dneg' = -dneg/tau = (q0 - shiftH q0) + (q1 - shiftW q1)
                # strip-0 of the H-part + the q0+q1 base via PE:
                #   psum0 = I@q1_s0 + I@q0_s0 + Sdm@q0_s3   (Sdm = -eye(k=1))
                # q1 matmul first: p1 is written before p0 at the end of the
                # previous iteration, so the PE chain starts earlier.
                nc.tensor.matmul(psum0[:], Ia[:], p1[:, 0:WT], start=True, stop=False)
                nc.tensor.matmul(psum0[:], Ia[:], p0[:, 0:WT], start=False, stop=False)
                nc.tensor.matmul(psum0[:], Sdm[:], p0[:, 3 * WT:], start=False, stop=True)
                # strips 1-3 base on DVE; strip 0 from PSUM via ACT
                nc.vector.tensor_add(d3[:, 1:4, :], p03[:, 1:4, :], p13[:, 1:4, :])
                nc.scalar.activation(d3[:, 0, :], psum0[:], ACTF.Copy)
                nc.vector.tensor_tensor(d3[:, 1:4, :], d3[:, 1:4, :],
                                        p03[:, 0:3, :], ALU.subtract)
                nc.vector.tensor_tensor(d3[:, :, 1:WT], d3[:, :, 1:WT],
                                        p13[:, :, 0:WT - 1], ALU.subtract)

                # Ed_j = sum((tau*dneg')^2) = sum(dneg^2)
                nc.scalar.activation(scr[:], dneg[:], ACTF.Square, scale=float(TAU),
                                     accum_out=eden[:, 2 * j:2 * j + 1])

                # t = img + tau*dneg'tile  (dneg'tile = -dneg/tau)
                nc.vector.tensor_scalar(tscl[:], dneg[:], float(TAU), None, ALU.mult)
                nc.vector.tensor_add(t[:], img[:], tscl[:])

            # strip-boundary block of g0: psum3 = Su@t_s0 - Iz@t_s3
            nc.tensor.matmul(psum3[:], Sup[:], t[:, 0:WT], start=True, stop=False)
            nc.tensor.matmul(psum3[:], Inz[:], t[:, 3 * WT:], start=False, stop=True)

            # g0 interior; boundary from PSUM
            nc.vector.tensor_tensor(g03[:, 0:3, :], t3[:, 1:4, :],
                                    t3[:, 0:3, :], ALU.subtract)
            nc.scalar.activation(g03[:, 3, :], psum3[:], ACTF.Copy)

            # g1 = shiftW^-1(t) - t  (col WT-1 stays 0)
            nc.vector.tensor_tensor(g13[:, :, 0:WT - 1], t3[:, :, 1:WT],
                                    t3[:, :, 0:WT - 1], ALU.subtract)

            # n2 = g0^2 + g1^2: sq0 on ACT (off-chain), sq1 on DVE (on-chain).
            # n2 is double-buffered: the previous iteration's off-chain
            # En-sqrt still reads the old buffer (avoids a WAR stall).
            n2 = n2a if j % 2 == 0 else n2b
            nc.scalar.activation(sq0[:], g0[:], ACTF.Square)
            nc.vector.tensor_mul(n2[:], g1[:], g1[:])

            HF = FREE // 2
            ha = (slice(None), slice(0, HF))
            hb = (slice(None), slice(HF, FREE))
            # halved r-chain interleaved with the u adds: sqrt_ha fires right
            # after n2add_ha while the DVE chews on u1/n2add_hb/u0
            nc.vector.tensor_add(n2[ha], n2[ha], sq0[ha])
            if j + 1 < K:
                nc.scalar.activation(s32[ha], n2[ha], ACTF.Sqrt)
                if j > 0:
                    nc.vector.tensor_add(u1[:], p1[:], g1[:])
            nc.vector.tensor_add(n2[hb], n2[hb], sq0[hb])
            if j + 1 < K:
                nc.scalar.activation(s32[hb], n2[hb], ACTF.Sqrt)
                if j > 0:
                    nc.vector.tensor_add(u0[:], p0[:], g0[:])
            # En_j = sum(norm): separate off-chain op so nothing waits on the
            # accumulator read
            nc.scalar.activation(scr[:], n2[:], ACTF.Sqrt,
                                 accum_out=eden[:, 2 * j + 1:2 * j + 2])

            if j + 1 < K:
                # r = 1 / (1 + (tau/weight)*norm), in pipelined halves.
                # The recip writes fp16 directly (the fp32 bit-trick is on the
                # INPUT; the output conversion is the normal DVE write path),
                # which removes the cast op.
                from concourse.dve_ops import (RECIP_APPROX_FAST_CONSTS,
                                               RECIPROCAL_APPROX_FAST)
                c = RECIP_APPROX_FAST_CONSTS
                for h in (ha, hb):
                    nc.vector.tensor_scalar(d32[h], s32[h], float(C_TW), 1.0,
                                            ALU.mult, ALU.add)
                    nc.vector._custom_dve(RECIPROCAL_APPROX_FAST, out=r[h],
                                          in0=d32[h], s0=c["s0"], s1=c["s1"],
                                          imm2=c["imm2"])
                # p1 first so the next iteration's d-chain starts earlier
                nc.vector.tensor_mul(p1[:], u1[:] if j > 0 else g1[:], r[:])
                nc.vector.tensor_mul(p0[:], u0[:] if j > 0 else g0[:], r[:])

            if j >= J_LO:
                # 4 chunks land on different HW DMA queues -> ~4x faster drain,
                # which matters for the last snapshot (nothing hides its tail)
                base = (j - J_LO) * FREE
                for jj in range(4):
                    nc.sync.dma_start(
                        ts_d.ap()[:, base + jj * WT:base + (jj + 1) * WT],
                        t[:, jj * WT:(jj + 1) * WT])

        nc.sync.dma_start(eden_d.ap(), eden[:])

    nc.compile()
    return nc


def _get_nc():
    global _NC
    if _NC is None:
        _NC = _build()
    return _NC


def _host_reference_fallback(img):
    """Exact CPU port of the reference (incl. freeze); only used if the
    device E-sequence fails to locate i* inside [J_LO, K)."""
    out = np.empty_like(img)
    for c in range(img.shape[0]):
        image = img[c].astype(np.float64)
        Hh, Ww = image.shape
        tau = 0.25
        p = np.zeros((2, Hh, Ww))
        o = image.copy()
        E_init = None
        E_prev = None
        for i in range(200):
            d = -p.sum(0)
            d[1:, :] += p[0, :-1, :]
            d[:, 1:] += p[1, :, :-1]
            o = image + d
            gg0 = np.zeros_like(o); gg0[:-1] = o[1:] - o[:-1]
            gg1 = np.zeros_like(o); gg1[:, :-1] = o[:, 1:] - o[:, :-1]
            nrm = np.sqrt(gg0 * gg0 + gg1 * gg1)
            E = ((d * d).sum() + WEIGHT * nrm.sum()) / (Hh * Ww)
            if i == 0:
                E_init = E
            elif abs(E_prev - E) < EPS * E_init:
                break
            E_prev = E
            p = (p - tau * np.stack([gg0, gg1])) / (1.0 + C_TW * nrm[None])
        out[c] = o.astype(np.float32)
    return out


def kernel(img: np.ndarray) -> np.ndarray:
    from concourse.bass_utils import run_bass_kernel_spmd

    assert img.shape == (3, 512, 512) and img.dtype == np.float32
    nc = _get_nc()
    del LAST_RESULTS[:]

    Ia = np.eye(P, dtype=np.float16)
    Sdm = (-np.eye(P, k=1)).astype(np.float16)   # psum0[m] -= q0_s3[m-1]
    Sup = np.eye(P, k=-1, dtype=np.float16)      # psum3[m] += t_s0[m+1]
    Inz = (-np.eye(P)).astype(np.float16)
    Inz[P - 1, P - 1] = 0.0                      # g0 row 511 = 0

    # core -> (channel, col range of its 288-wide slice)
    col_lo = [0, H - WT]     # half 0: cols 0..287; half 1: cols 224..511
    core_map = [(c // 2, c % 2) for c in range(6)] + [(0, 0), (1, 0)]

    in_maps = []
    for c in range(N_CORES):
        ch, half = core_map[c]
        lo = col_lo[half]
        sl = np.ascontiguousarray(img[ch][:, lo:lo + WT]).astype(np.float16)
        in_maps.append({
            "img": sl.reshape(P, FREE),
            "Ia": Ia, "Sdm": Sdm, "Sup": Sup, "Inz": Inz,
        })

    res = run_bass_kernel_spmd(nc, in_maps, list(range(N_CORES)))
    LAST_RESULTS.append(res)
    outs = res.results

    result = np.empty((3, 512, 512), np.float32)
    ok = True
    for ch in range(3):
        # E_j from the pair's summed partials (scale-invariant stopping rule)
        ed = (outs[2 * ch]["eden"].astype(np.float64).sum(0)
              + outs[2 * ch + 1]["eden"].astype(np.float64).sum(0))
        edp = ed[0::2].copy()
        edp[0] = 0.0     # Ed_0 == 0 (p starts at 0); col 0 is never written
        E = edp + WEIGHT * ed[1::2]
        istar = None
        for jj in range(1, K):
            if abs(E[jj - 1] - E[jj]) < EPS * E[0]:
                istar = jj
                break
        DIAG[ch] = (istar, [abs(E[jj - 1] - E[jj]) / (EPS * E[0])
                            for jj in range(1, K)])
        if istar is None or istar < J_LO:
            ok = False
            break
        for half in (0, 1):
            t = outs[2 * ch + half]["ts"][:, (istar - J_LO) * FREE:
                                          (istar - J_LO + 1) * FREE]
            t = t.reshape(H, WT).astype(np.float32)
            if half == 0:
                result[ch][:, 0:OWN] = t[:, 0:OWN]
            else:
                result[ch][:, OWN:H] = t[:, WT - OWN:WT]
    if not ok:
        return _host_reference_fallback(img)
    return result
